# revision 1
# baseline (speedup 1.0000x reference)
"""Trainium2 Bass kernel for nn_AGNN_EFG (GCN -> TopK pool -> GATv2 -> TopK pool -> head).

Self-contained: shards the B=64 graphs across 8 NeuronCores (8 graphs/core),
runs one SPMD Bass program, gathers the [64, 1] head output on host.

Per-core algorithm (8 graphs x 2048 nodes, 64 feat), v2:
- Edge list reorganized per graph (original edges + self loops, graph-local
  int16 ids) so every stage runs per graph and pipelines across graphs.
- Degrees via a PE one-hot-matmul histogram (no DMA histogram pass):
  per 128-edge chunk, hist[hi, lo] += onehot16(dst>>7)^T @ onehot128(dst&127),
  accumulated in PSUM per graph; overlaps the GCN edge pass of prior graphs.
- TopKPooling as per-graph score thresholds (bisection in a transposed
  "t-space" [node%128, node//128] layout); output is order-invariant so no
  node compaction is ever needed (masking only).
- GCN aggregation: u = dinv*(x@W1) in a per-graph DRAM table (256B f32 rows),
  one dma_gather pass by src + one dma_scatter_add pass by dst.
- GATv2: per-graph packed [xl | xr] bf16 table (256B rows); dropped nodes'
  rows are NaN; per-edge e = att . leaky(xl_s + xr_d) on DVE; edges touching
  dropped nodes (NaN e) masked to -30 before exp and their xl zeroed; packed
  [w*xl | w] rows scatter-added into a numerator|denominator table.
"""

import sys

sys.path.insert(0, "/opt/trn_rl_repo")

from dataclasses import dataclass

import numpy as np
import ml_dtypes

import concourse.bass as bass
import concourse.mybir as mybir
import concourse.tile as tile
from concourse import bacc
from concourse.bass_utils import run_bass_kernel_spmd
from concourse.masks import make_identity

P = 128
F32 = mybir.dt.float32
BF16 = mybir.dt.bfloat16
I16 = mybir.dt.int16
U8 = mybir.dt.uint8
AF = mybir.ActivationFunctionType
OP = mybir.AluOpType
AX = mybir.AxisListType


@dataclass
class Cfg:
    ng: int = 8          # graphs per core
    npg: int = 2048      # nodes per graph
    hid: int = 64        # feature dim
    eg: int = 32768      # edges per graph (original, without self loops)
    ch: int = 1024       # edge chunk per gather/scatter call (>=2048 faults the SWDGE ring)
    n_bisect: int = 30   # bisection iterations for topk threshold
    probe: int = 0       # 0=full, 1=stop after GCN, 2=stop after pool1
    psign: float = -1.0  # -sign(sum(att)): poison sign so poisoned e < 0
    pb_mag: float = 200.0  # poison magnitude; set so poisoned e ~ -40 (LUT-safe)

    @property
    def nn(self):
        return self.ng * self.npg

    @property
    def ne(self):
        return self.ng * self.eg

    @property
    def epg(self):  # edges per graph incl self loops
        return self.eg + self.npg

    @property
    def ne2(self):
        return self.ng * self.epg

    @property
    def nt(self):
        return self.nn // P

    @property
    def tj(self):
        return self.npg // P

    @property
    def k1(self):
        return self.npg // 2

    @property
    def k2(self):
        return self.npg // 4

    @property
    def chunks(self):
        out = []
        off = 0
        while off < self.epg:
            c = min(self.ch, self.epg - off)
            assert c % P == 0
            out.append((off, c))
            off += c
        return out


def build_core_program(ctx, tc, cfg: Cfg):
    nc = tc.nc
    NG, NPG, HID, NN = cfg.ng, cfg.npg, cfg.hid, cfg.nn
    CH, NT, TJ, EPG = cfg.ch, cfg.nt, cfg.tj, cfg.epg
    CHP = CH // P
    EW = EPG // 16    # wrapped idx cols per graph
    EPM = EPG // P    # payload-major cols per graph
    assert HID == 64 and NPG % P == 0 and EPG % 16 == 0 and EPG % P == 0

    # ---- I/O ----
    xT = nc.dram_tensor("xT", [HID, NN], F32, kind="ExternalInput").ap()
    srcw = nc.dram_tensor("srcw", [P, NG * EW], I16, kind="ExternalInput").ap()
    dstw = nc.dram_tensor("dstw", [P, NG * EW], I16, kind="ExternalInput").ap()
    dlo_d = nc.dram_tensor("dlo", [P, NG * EPM], BF16, kind="ExternalInput").ap()
    dhi_d = nc.dram_tensor("dhi", [P, NG * EPM], BF16, kind="ExternalInput").ap()
    w_names = ["W1", "Wlin1", "Wl", "Wr"]
    Wd = {n: nc.dram_tensor(n, [HID, HID], F32, kind="ExternalInput").ap() for n in w_names}
    v_names = ["b1", "bn_a", "bn_b", "b_lin1", "p1", "att", "b_gat", "p2", "W23"]
    Vd = {n: nc.dram_tensor(n, [HID], F32, kind="ExternalInput").ap() for n in v_names}
    Cd = nc.dram_tensor("Cc", [1], F32, kind="ExternalInput").ap()
    out_d = nc.dram_tensor("out", [NG, 1], F32, kind="ExternalOutput").ap()

    # ---- DRAM scratch (per core; edge lists are window-packed so each
    # scatter call has distinct destination rows -> no RMW races) ----
    utab = nc.dram_tensor("utab", [NN, HID], F32).ap()
    htabs = [nc.dram_tensor(f"htab{i}", [NN, HID], F32).ap() for i in range(4)]
    gtab = nc.dram_tensor("gtab", [NN, P], BF16).ap()
    h2dens = [nc.dram_tensor(f"h2den{i}", [NN, P], BF16).ap() for i in range(4)]
    sc_dram = nc.dram_tensor("sc_dram", [NN], F32).ap()

    cpool = ctx.enter_context(tc.tile_pool(name="consts", bufs=1))
    mpool = ctx.enter_context(tc.tile_pool(name="main", bufs=1))
    ppool = ctx.enter_context(tc.tile_pool(name="psum", bufs=1, space="PSUM"))

    # ---- constants ----
    ident = cpool.tile([P, P], F32)
    make_identity(nc, ident[:])
    ones128 = cpool.tile([P, P], F32)
    nc.gpsimd.memset(ones128[:], 1.0)
    nantile = cpool.tile([P, NT], F32)
    nc.gpsimd.memset(nantile[:], float("nan"))
    negbig = cpool.tile([P, NT], F32)
    nc.gpsimd.memset(negbig[:], -1e9)
    io16 = cpool.tile([P, P], I16)
    nc.gpsimd.iota(io16[:], pattern=[[1, P]], base=0, channel_multiplier=0)
    iota_bf = cpool.tile([P, P], BF16)
    nc.vector.tensor_copy(out=iota_bf[:], in_=io16[:])
    zbf = cpool.tile([P, HID], BF16)
    nc.gpsimd.memset(zbf[:], 0.0)

    Ws = {}
    for n in w_names:
        t = cpool.tile([HID, HID], F32, tag=f"w_{n}")
        nc.sync.dma_start(out=t[:], in_=Wd[n][:])
        Ws[n] = t
    Vs = {}
    for n in v_names:
        t = cpool.tile([HID, 1], F32, tag=f"v_{n}")
        nc.sync.dma_start(out=t[:], in_=Vd[n][:, None])
        Vs[n] = t
    att_rep = cpool.tile([P, HID], F32)
    nc.sync.dma_start(out=att_rep[:], in_=Vd["att"][None, :].to_broadcast([P, HID]))
    p2_rep = cpool.tile([P, HID], F32)
    nc.sync.dma_start(out=p2_rep[:], in_=Vd["p2"][None, :].to_broadcast([P, HID]))
    bgat_rep = cpool.tile([P, HID], F32)
    nc.sync.dma_start(out=bgat_rep[:], in_=Vd["b_gat"][None, :].to_broadcast([P, HID]))
    Cc_sb = cpool.tile([NG, 1], F32)
    nc.sync.dma_start(out=Cc_sb[:], in_=Cd[None, :].to_broadcast([NG, 1]))

    zt = cpool.tile([P, 512], F32)
    nc.gpsimd.memset(zt[:], 0.0)

    def zero_tab(tab_ap, total_els_f32):
        flat = tab_ap.bitcast(F32).rearrange("a b -> (a b)")
        step = P * 512
        for o in range(0, total_els_f32, step):
            w = min(step, total_els_f32 - o)
            nc.sync.dma_start(
                out=flat[o : o + w].rearrange("(p f) -> p f", p=P),
                in_=zt[:, : w // P],
            )

    # histogram inputs (whole tables in SBUF)
    dlo_sb = cpool.tile([P, NG * EPM], BF16)
    nc.sync.dma_start(out=dlo_sb[:], in_=dlo_d[:])
    dhi_sb = cpool.tile([P, NG * EPM], BF16)
    nc.sync.dma_start(out=dhi_sb[:], in_=dhi_d[:])

    dinv_t = mpool.tile([P, NT], F32, tag="dinv_t")
    sqd_t = mpool.tile([P, NT], F32, tag="sqd_t")
    ntmp = mpool.tile([P, NT], F32, tag="ntmp")

    def recip_newton(r_ap, x_ap, tmp_ap):
        nc.vector.tensor_tensor(out=tmp_ap, in0=x_ap, in1=r_ap, op=OP.mult)
        nc.vector.tensor_scalar(
            out=tmp_ap, in0=tmp_ap, scalar1=-1.0, scalar2=2.0, op0=OP.mult, op1=OP.add
        )
        nc.vector.tensor_tensor(out=r_ap, in0=r_ap, in1=tmp_ap, op=OP.mult)

    NWIN = cfg.ne2 // CH
    assert cfg.ne2 % CH == 0

    def idx_slice(pool, table, k, tag):
        t = pool.tile([P, CH // 16], I16, tag=tag)
        c0 = k * (CH // 16)
        nc.sync.dma_start(out=t[:], in_=table[:, c0 : c0 + CH // 16])
        return t

    # ======== per-graph: histogram -> dinv -> u-table -> GCN edge pass ========
    from contextlib import ExitStack as _ES
    phase1 = _ES()
    hist_pool = phase1.enter_context(tc.tile_pool(name="histp", bufs=2, space="PSUM"))
    hsp = phase1.enter_context(tc.tile_pool(name="hists", bufs=2))
    s3p = phase1.enter_context(tc.tile_pool(name="s3p", bufs=1, space="PSUM"))
    s3s = phase1.enter_context(tc.tile_pool(name="s3s", bufs=2))
    gcnp = phase1.enter_context(tc.tile_pool(name="gcn_e", bufs=6))

    do_hist = cfg.probe != 6
    do_gcn = cfg.probe != 5
    if cfg.probe == 6:
        nc.gpsimd.memset(dinv_t[:], 1.0)
    for g in range(NG):
        if not do_hist:
            break
        # ---- histogram of dst (incl self loops) => deg in t-space ----
        hist_ps = hist_pool.tile([16, P], F32, tag="histps", name=f"histps{g}")
        q0 = g * EPM
        first = True
        for b0 in range(0, EPM, 8):
            nb8 = min(8, EPM - b0)
            ohL = hsp.tile([P, 8, P], BF16, tag="ohL", name=f"ohL{g}_{b0}")
            ohS = hsp.tile([P, 8, 16], BF16, tag="ohS", name=f"ohS{g}_{b0}")
            nc.vector.tensor_tensor(
                out=ohL[:, :nb8, :],
                in0=iota_bf[:, None, :].to_broadcast([P, nb8, P]),
                in1=dlo_sb[:, q0 + b0 : q0 + b0 + nb8, None].to_broadcast([P, nb8, P]),
                op=OP.is_equal,
            )
            nc.vector.tensor_tensor(
                out=ohS[:, :nb8, :],
                in0=iota_bf[:, None, :16].to_broadcast([P, nb8, 16]),
                in1=dhi_sb[:, q0 + b0 : q0 + b0 + nb8, None].to_broadcast([P, nb8, 16]),
                op=OP.is_equal,
            )
            for k in range(nb8):
                nc.tensor.matmul(
                    hist_ps[:],
                    lhsT=ohS[:, k, :],
                    rhs=ohL[:, k, :],
                    start=first,
                    stop=(b0 + 8 >= EPM and k == nb8 - 1),
                )
                first = False
        hist_sb = hsp.tile([16, P], F32, tag="hist_sb", name=f"hist_sb{g}")
        nc.vector.tensor_copy(out=hist_sb[:], in_=hist_ps[:])
        histT = hist_pool.tile([P, 16], F32, tag="histT", name=f"histT{g}")
        nc.tensor.transpose(out=histT[:], in_=hist_sb[:], identity=ident[:16, :16])
        gsl = slice(g * TJ, (g + 1) * TJ)
        nc.scalar.sqrt(out=sqd_t[:, gsl], in_=histT[:, :TJ])
        nc.vector.reciprocal(out=dinv_t[:, gsl], in_=sqd_t[:, gsl])
        recip_newton(dinv_t[:, gsl], sqd_t[:, gsl], ntmp[:, gsl])

    for g in range(NG):
        if not do_gcn:
            break
        # ---- u-table: dinv * transpose(x @ W1) ----
        xTg = s3s.tile([HID, NPG], F32, tag="xTg", name=f"xTg{g}")
        nc.sync.dma_start(out=xTg[:], in_=xT[:, g * NPG : (g + 1) * NPG])
        u_stage = s3s.tile([P, TJ, HID], F32, tag="ustage", name=f"ustage{g}")
        for jl in range(TJ):
            j = g * TJ + jl
            pm = s3p.tile([HID, P], F32, tag="xwps", name=f"xwps{g}_{jl}")
            nc.tensor.matmul(
                pm[:], lhsT=Ws["W1"][:], rhs=xTg[:, jl * P : (jl + 1) * P],
                start=True, stop=True,
            )
            xw_t = s3s.tile([HID, P], F32, tag="xwsb", name=f"xwsb{g}_{jl}")
            nc.scalar.copy(out=xw_t[:], in_=pm[:])
            pt = s3p.tile([P, HID], F32, tag="trps", name=f"trps{g}_{jl}")
            nc.tensor.transpose(out=pt[:], in_=xw_t[:], identity=ident[:HID, :HID])
            nc.vector.tensor_scalar(
                out=u_stage[:, jl, :], in0=pt[:], scalar1=dinv_t[:, j : j + 1],
                scalar2=None, op0=OP.mult,
            )
        nc.sync.dma_start(
            out=utab.rearrange("(j p) f -> p j f", p=P)[:, g * TJ : (g + 1) * TJ, :],
            in_=u_stage[:],
        )

    # ---- GCN edge pass (window-packed; all graphs) ----
    for t in htabs:
        zero_tab(t, NN * HID)
    for k in range(NWIN):
        ssl = idx_slice(gcnp, srcw, k, "ssl")
        dsl = idx_slice(gcnp, dstw, k, "dsl")
        ub = gcnp.tile([P, CHP, HID], F32, tag="ubuf")
        nc.gpsimd.dma_gather(
            out_ap=ub[:], in_ap=utab[:], idxs_ap=ssl[:],
            num_idxs=CH, num_idxs_reg=CH, elem_size=HID, queue_num=k % 4,
        )
        nc.gpsimd.dma_scatter_add(
            out_ap=htabs[k % 4][:], in_ap=ub[:], idxs_ap=dsl[:],
            num_idxs=CH, num_idxs_reg=CH, elem_size=HID, queue_num=k % 4,
        )

    phase1.close()

    if cfg.probe in (5, 6):
        pr = mpool.tile([NG, 1], F32, tag="probe_o")
        if cfg.probe == 5:
            nc.vector.tensor_copy(out=pr[:], in_=dinv_t[0:NG, 0:1])
            nc.sync.dma_start(out=out_d[:], in_=pr[:])
        else:
            nc.sync.dma_start(out=pr[:], in_=htabs[0][0:NG, 0:1])
            nc.sync.dma_start(out=out_d[:], in_=pr[:])
        return

    if cfg.probe == 1:
        pr = mpool.tile([NG, 1], F32, tag="probe_o")
        nc.sync.dma_start(out=pr[:], in_=htabs[0][0:NG, 0:1])
        nc.sync.dma_start(out=out_d[:], in_=pr[:])
        return

    # ======== per graph: h readback -> dense BN/leaky -> lin1 -> scores/xlr ========
    xlr = mpool.tile([P, NN], BF16, tag="bigC2")
    S6C = min(512, NPG)
    phase2 = _ES()
    s5s = phase2.enter_context(tc.tile_pool(name="s5s", bufs=2))
    s5p = phase2.enter_context(tc.tile_pool(name="s5p", bufs=2, space="PSUM"))
    s6s = phase2.enter_context(tc.tile_pool(name="s6s", bufs=2))
    s6p = phase2.enter_context(tc.tile_pool(name="s6p", bufs=1, space="PSUM"))
    for g in range(NG):
        h_stage = s5s.tile([P, TJ, HID], F32, tag="hstage", name=f"hstage{g}")
        h_stage2 = s5s.tile([P, TJ, HID], F32, tag="hstage2", name=f"hstage2_{g}")
        h_stage3 = s5s.tile([P, TJ, HID], F32, tag="hstage3", name=f"hstage3_{g}")
        h_stage4 = s5s.tile([P, TJ, HID], F32, tag="hstage4", name=f"hstage4_{g}")
        for t, tab in zip((h_stage, h_stage2, h_stage3, h_stage4), htabs):
            nc.sync.dma_start(
                out=t[:],
                in_=tab.rearrange("(j p) f -> p j f", p=P)[:, g * TJ : (g + 1) * TJ, :],
            )
        nc.vector.tensor_tensor(
            out=h_stage[:], in0=h_stage[:], in1=h_stage2[:], op=OP.add
        )
        nc.vector.tensor_tensor(
            out=h_stage3[:], in0=h_stage3[:], in1=h_stage4[:], op=OP.add
        )
        nc.vector.tensor_tensor(
            out=h_stage[:], in0=h_stage[:], in1=h_stage3[:], op=OP.add
        )
        hfm = s5s.tile([HID, NPG], F32, tag="hfm", name=f"hfm{g}")
        for jl in range(TJ):
            j = g * TJ + jl
            hs = s5s.tile([P, HID], F32, tag="hs", name=f"hs{g}_{jl}")
            nc.vector.tensor_scalar(
                out=hs[:], in0=h_stage[:, jl, :], scalar1=dinv_t[:, j : j + 1],
                scalar2=None, op0=OP.mult,
            )
            pt = s5p.tile([HID, P], F32, tag="trps", name=f"htr{g}_{jl}")
            nc.tensor.transpose(out=pt[:], in_=hs[:], identity=ident[:])
            nc.scalar.copy(out=hfm[:, jl * P : (jl + 1) * P], in_=pt[:])
        nc.vector.tensor_scalar(
            out=hfm[:], in0=hfm[:], scalar1=Vs["b1"][:], scalar2=None, op0=OP.add
        )
        nc.vector.tensor_scalar(
            out=hfm[:], in0=hfm[:], scalar1=Vs["bn_a"][:], op0=OP.mult,
            scalar2=Vs["bn_b"][:], op1=OP.add,
        )
        nc.vector.scalar_tensor_tensor(
            out=hfm[:], in0=hfm[:], scalar=0.01, in1=hfm[:], op0=OP.mult, op1=OP.max,
        )
        for ol in range(0, NPG, S6C):
            o = g * NPG + ol
            pm = s6p.tile([HID, S6C], F32, tag="l1ps", name=f"l1ps{o}")
            nc.tensor.matmul(
                pm[:], lhsT=Ws["Wlin1"][:], rhs=hfm[:, ol : ol + S6C],
                start=True, stop=True,
            )
            hc = s6s.tile([HID, S6C], F32, tag="hc", name=f"hc{o}")
            nc.scalar.activation(
                out=hc[:], in_=pm[:], func=AF.Identity, bias=Vs["b_lin1"][:]
            )
            ps = s6p.tile([1, S6C], F32, tag="scps", name=f"scps{o}")
            nc.tensor.matmul(ps[:], lhsT=Vs["p1"][:], rhs=hc[:], start=True, stop=True)
            sc = s6s.tile([1, S6C], F32, tag="scsb", name=f"scsb{o}")
            nc.vector.tensor_copy(out=sc[:], in_=ps[:])
            nc.sync.dma_start(out=sc_dram[None, o : o + S6C], in_=sc[:])
            px = s6p.tile([P, S6C], F32, tag="xlrps", name=f"xlrps{o}")
            nc.tensor.matmul(px[:HID, :], lhsT=Ws["Wl"][:], rhs=hc[:], start=True, stop=True)
            nc.tensor.matmul(px[HID:, :], lhsT=Ws["Wr"][:], rhs=hc[:], start=True, stop=True)
            nc.scalar.copy(out=xlr[:, o : o + S6C], in_=px[:])

    phase2.close()
    score1_t = mpool.tile([P, NT], F32, tag="score1_t")
    nc.sync.dma_start(out=score1_t[:], in_=sc_dram.rearrange("(j p) -> p j", p=P))

    # ---- bisection: per-graph threshold so that #(score > t) == target ----
    def bisect_threshold(score_t, target, tag):
        lo = mpool.tile([P, NG], F32, tag=f"lo_{tag}", name=f"lo_{tag}")
        hi = mpool.tile([P, NG], F32, tag=f"hi_{tag}", name=f"hi_{tag}")
        mid = mpool.tile([P, NG], F32, tag=f"mid_{tag}", name=f"mid_{tag}")
        cmp = mpool.tile([P, NT], F32, tag=f"cmp_{tag}", name=f"cmp_{tag}")
        cred = mpool.tile([P, NG], F32, tag=f"cred_{tag}", name=f"cred_{tag}")
        ge = mpool.tile([P, NG], U8, tag=f"ge_{tag}", name=f"ge_{tag}")
        lt = mpool.tile([P, NG], U8, tag=f"lt_{tag}", name=f"lt_{tag}")
        nc.gpsimd.memset(lo[:], -64.0)
        nc.gpsimd.memset(hi[:], 64.0)
        sc_g = score_t[:].rearrange("p (g t) -> p g t", g=NG)
        cmp_g = cmp[:].rearrange("p (g t) -> p g t", g=NG)
        for it in range(cfg.n_bisect):
            nc.vector.tensor_tensor(out=mid[:], in0=lo[:], in1=hi[:], op=OP.add)
            nc.vector.tensor_scalar(
                out=mid[:], in0=mid[:], scalar1=0.5, scalar2=None, op0=OP.mult
            )
            nc.vector.tensor_tensor(
                out=cmp_g, in0=sc_g,
                in1=mid[:, :, None].to_broadcast([P, NG, TJ]), op=OP.is_gt,
            )
            nc.vector.tensor_reduce(out=cred[:], in_=cmp_g, axis=AX.X, op=OP.add)
            cps = ppool.tile([P, NG], F32, tag="pp", name=f"cnt_{tag}_{it}")
            nc.tensor.matmul(cps[:], lhsT=ones128[:], rhs=cred[:], start=True, stop=True)
            nc.vector.tensor_scalar(
                out=ge[:], in0=cps[:], scalar1=float(target), scalar2=None, op0=OP.is_ge
            )
            nc.vector.tensor_scalar(
                out=lt[:], in0=cps[:], scalar1=float(target), scalar2=None, op0=OP.is_lt
            )
            nc.vector.copy_predicated(out=lo[:], mask=ge[:], data=mid[:])
            nc.vector.copy_predicated(out=hi[:], mask=lt[:], data=mid[:])
        return lo

    # ======== pool1 threshold; gate1 = tanh(score) kept else NaN ========
    t1 = bisect_threshold(score1_t, cfg.k1, "p1")
    kept1 = mpool.tile([P, NT], U8, tag="kept1")
    nc.vector.tensor_tensor(
        out=kept1[:].rearrange("p (g t) -> p g t", g=NG),
        in0=score1_t[:].rearrange("p (g t) -> p g t", g=NG),
        in1=t1[:, :, None].to_broadcast([P, NG, TJ]), op=OP.is_gt,
    )
    tanh1 = mpool.tile([P, NT], F32, tag="tanh1")
    nc.scalar.activation(out=tanh1[:], in_=score1_t[:], func=AF.Tanh)
    gate1 = mpool.tile([P, NT], F32, tag="gate1")
    nc.vector.tensor_copy(out=gate1[:], in_=nantile[:])
    nc.vector.copy_predicated(out=gate1[:], mask=kept1[:], data=tanh1[:])
    # zero-gate + additive poison: dropped rows of gtab become psign*PB so
    # every edge touching them gets e = -0.2*PB*|sum(att)| -> w = exp = 0.
    gate1z = mpool.tile([P, NT], F32, tag="gate1z")
    nc.vector.memset(gate1z[:], 0.0)
    nc.vector.copy_predicated(out=gate1z[:], mask=kept1[:], data=tanh1[:])
    padd = mpool.tile([P, NT], F32, tag="padd")
    pb = cfg.psign * cfg.pb_mag
    nc.vector.tensor_scalar(
        out=padd[:], in0=kept1[:], scalar1=-pb, scalar2=pb, op0=OP.mult, op1=OP.add
    )

    if cfg.probe == 2:
        pr2 = mpool.tile([NG, 1], F32, tag="probe_o")
        nc.vector.tensor_copy(out=pr2[:], in_=t1[0:NG, 0:1])
        nc.sync.dma_start(out=out_d[:], in_=pr2[:])
        return

    # ======== gtab = gate1 * transpose(xlr)  (NaN rows for dropped) ========
    xlr_dram = nc.dram_tensor("xlr_dram", [P, NN], BF16).ap()
    nc.sync.dma_start(out=xlr_dram[:], in_=xlr[:])
    gstage = mpool.tile([P, NT, P], BF16, tag="bigC")
    nc.sync.dma_start_transpose(out=gstage[:], in_=xlr_dram[:])
    nc.vector.tensor_tensor(
        out=gstage[:], in0=gstage[:],
        in1=gate1z[:, :, None].to_broadcast([P, NT, P]), op=OP.mult,
    )
    nc.vector.tensor_tensor(
        out=gstage[:], in0=gstage[:],
        in1=padd[:, :, None].to_broadcast([P, NT, P]), op=OP.add,
    )
    nc.sync.dma_start(out=gtab.rearrange("(j p) f -> p j f", p=P), in_=gstage[:])

    # ======== GAT edge pass (per graph) ========
    gat_pay = [cpool.tile([P, CHP, P], BF16, tag=f"gpay{i}", name=f"gpay{i}") for i in range(4)]
    for t in gat_pay:
        nc.gpsimd.memset(t[:], 0.0)
    phase4 = _ES()
    gate_p = phase4.enter_context(tc.tile_pool(name="gat_e", bufs=4))
    gatt = phase4.enter_context(tc.tile_pool(name="gat_t", bufs=1))
    ci = 0
    for t in h2dens:
        zero_tab(t, NN * P // 2)
    if True:
        for k in range(NWIN):
            clp = CHP
            cl = CH
            ssl = idx_slice(gate_p, srcw, k, "ssl")
            dsl = idx_slice(gate_p, dstw, k, "dsl")
            gS = gate_p.tile([P, CHP, P], BF16, tag="gS")
            gD = gate_p.tile([P, CHP, P], BF16, tag="gD")
            nc.gpsimd.dma_gather(
                out_ap=gS[:, :clp, :], in_ap=gtab[:], idxs_ap=ssl[:, : cl // 16],
                num_idxs=cl, num_idxs_reg=cl, elem_size=P, queue_num=k % 4,
            )
            nc.gpsimd.dma_gather(
                out_ap=gD[:, :clp, :], in_ap=gtab[:], idxs_ap=dsl[:, : cl // 16],
                num_idxs=cl, num_idxs_reg=cl, elem_size=P, queue_num=(k + 1) % 4,
            )
            z = mpool.tile([P, CHP, HID], BF16, tag="bigC2", name=f"z_{ci}")
            nc.vector.tensor_tensor(
                out=z[:, :clp, :], in0=gS[:, :clp, :HID], in1=gD[:, :clp, HID:], op=OP.add
            )
            nc.vector.scalar_tensor_tensor(
                out=z[:, :clp, :], in0=z[:, :clp, :], scalar=0.2, in1=z[:, :clp, :],
                op0=OP.mult, op1=OP.max,
            )
            prod = mpool.tile([P, CHP, HID], F32, tag="bigC", name=f"prod_{ci}")
            nc.vector.tensor_tensor(
                out=prod[:, :clp, :], in0=z[:, :clp, :],
                in1=att_rep[:, None, :].to_broadcast([P, clp, HID]), op=OP.mult,
            )
            eb = gatt.tile([P, CHP], F32, tag="eb")
            nc.vector.tensor_reduce(
                out=eb[:, :clp], in_=prod[:, :clp, :], axis=AX.X, op=OP.add
            )
            pay = gat_pay[ci % 4]
            nc.scalar.activation(
                out=pay[:, :clp, HID : HID + 1], in_=eb[:, :clp, None], func=AF.Exp
            )
            nc.vector.tensor_tensor(
                out=pay[:, :clp, :HID], in0=gS[:, :clp, :HID],
                in1=pay[:, :clp, HID : HID + 1].to_broadcast([P, clp, HID]),
                op=OP.mult,
            )
            nc.gpsimd.dma_scatter_add(
                out_ap=h2dens[k % 4][:], in_ap=pay[:, :clp, :], idxs_ap=dsl[:, : cl // 16],
                num_idxs=cl, num_idxs_reg=cl, elem_size=P, queue_num=k % 4,
            )
            ci += 1

    phase4.close()

    if cfg.probe == 3:
        pr3 = mpool.tile([NG, 1], F32, tag="probe_o")
        h2p = mpool.tile([NG, 2], BF16, tag="probe_b")
        nc.sync.dma_start(out=h2p[:], in_=h2dens[0][0:NG, 0:2])
        nc.vector.tensor_copy(out=pr3[:], in_=h2p[:, 0:1])
        nc.sync.dma_start(out=out_d[:], in_=pr3[:])
        return

    # ======== h2 = leaky(numer/denom + b_gat) in node-major tiles ========
    h2nm = mpool.tile([P, NT, P], BF16, tag="bigC2")
    h2nm2 = mpool.tile([P, NT, P], BF16, tag="bigC")
    nc.sync.dma_start(out=h2nm[:], in_=h2dens[0].rearrange("(j p) f -> p j f", p=P))
    nc.sync.dma_start(out=h2nm2[:], in_=h2dens[1].rearrange("(j p) f -> p j f", p=P))
    nc.vector.tensor_tensor(out=h2nm[:], in0=h2nm[:], in1=h2nm2[:], op=OP.add)
    nc.sync.dma_start(out=h2nm2[:], in_=h2dens[2].rearrange("(j p) f -> p j f", p=P))
    nc.vector.tensor_tensor(out=h2nm[:], in0=h2nm[:], in1=h2nm2[:], op=OP.add)
    nc.sync.dma_start(out=h2nm2[:], in_=h2dens[3].rearrange("(j p) f -> p j f", p=P))
    nc.vector.tensor_tensor(out=h2nm[:], in0=h2nm[:], in1=h2nm2[:], op=OP.add)
    rec_t = mpool.tile([P, NT], F32, tag="rec_t")
    den_t = mpool.tile([P, NT], F32, tag="den_t")
    ntmp2 = mpool.tile([P, NT], F32, tag="ntmp")
    nc.vector.tensor_scalar(
        out=den_t[:], in0=h2nm[:, :, HID], scalar1=1e-16, scalar2=None, op0=OP.add
    )
    nc.vector.reciprocal(out=rec_t[:], in_=den_t[:])
    recip_newton(rec_t[:], den_t[:], ntmp2[:])
    h2t = mpool.tile([P, NT, HID], F32, tag="bigA")
    nc.vector.tensor_tensor(
        out=h2t[:], in0=h2nm[:, :, :HID],
        in1=rec_t[:, :, None].to_broadcast([P, NT, HID]), op=OP.mult,
    )
    nc.vector.tensor_tensor(
        out=h2t[:], in0=h2t[:],
        in1=bgat_rep[:, None, :].to_broadcast([P, NT, HID]), op=OP.add,
    )
    nc.vector.scalar_tensor_tensor(
        out=h2t[:], in0=h2t[:], scalar=0.01, in1=h2t[:], op0=OP.mult, op1=OP.max
    )

    # ======== score2 (t-space), mask to kept1 ========
    score2_t = mpool.tile([P, NT], F32, tag="score2_t")
    s2prod = mpool.tile([P, NT, HID], F32, tag="bigC", name="s2prod")
    nc.vector.tensor_tensor(
        out=s2prod[:], in0=h2t[:],
        in1=p2_rep[:, None, :].to_broadcast([P, NT, HID]), op=OP.mult,
    )
    nc.vector.tensor_reduce(out=score2_t[:], in_=s2prod[:], axis=AX.X, op=OP.add)
    kept1_t = mpool.tile([P, NT], U8, tag="kept1_t")
    nc.vector.tensor_tensor(out=kept1_t[:], in0=gate1[:], in1=gate1[:], op=OP.is_equal)
    sc2m = mpool.tile([P, NT], F32, tag="sc2m")
    nc.vector.tensor_copy(out=sc2m[:], in_=negbig[:])
    nc.vector.copy_predicated(out=sc2m[:], mask=kept1_t[:], data=score2_t[:])

    # ======== pool2 threshold + gate2 = tanh * mask ========
    t2 = bisect_threshold(sc2m, cfg.k2, "p2")
    gate2 = mpool.tile([P, NT], F32, tag="gate2")
    nc.vector.tensor_tensor(
        out=gate2[:].rearrange("p (g t) -> p g t", g=NG),
        in0=sc2m[:].rearrange("p (g t) -> p g t", g=NG),
        in1=t2[:, :, None].to_broadcast([P, NG, TJ]), op=OP.is_gt,
    )
    tanh2 = mpool.tile([P, NT], F32, tag="tanh2")
    sc2c = mpool.tile([P, NT], F32, tag="sc2c")
    nc.vector.tensor_scalar(
        out=sc2c[:], in0=sc2m[:], scalar1=-64.0, scalar2=None, op0=OP.max
    )
    nc.scalar.activation(out=tanh2[:], in_=sc2c[:], func=AF.Tanh)
    nc.vector.tensor_tensor(out=gate2[:], in0=gate2[:], in1=tanh2[:], op=OP.mult)

    # ======== T_g = sum_n gate2[n] * h2[n]; out = T @ W23 + C ========
    Tps = ppool.tile([P, NG], F32, tag="pp2")
    for j in range(NT):
        g = j // TJ
        nc.tensor.matmul(
            Tps[:HID, g : g + 1], lhsT=h2t[:, j, :], rhs=gate2[:, j : j + 1],
            start=(j % TJ == 0), stop=(j % TJ == TJ - 1),
        )
    Tsb = mpool.tile([HID, NG], F32, tag="Tsb")
    nc.scalar.copy(out=Tsb[:], in_=Tps[:HID, :])
    hps = ppool.tile([NG, 1], F32, tag="pp")
    nc.tensor.matmul(hps[:], lhsT=Tsb[:], rhs=Vs["W23"][:], start=True, stop=True)
    outsb = mpool.tile([NG, 1], F32, tag="outsb")
    nc.vector.tensor_tensor(out=outsb[:], in0=hps[:], in1=Cc_sb[:], op=OP.add)
    nc.sync.dma_start(out=out_d[:], in_=outsb[:])


# ================= host side =================

def _wrap_idx(ix: np.ndarray) -> np.ndarray:
    n = ix.shape[0]
    w = ix.reshape(n // 16, 16).T.astype(np.int16)
    return np.tile(w, (8, 1)).copy()


def _pm_layout(v: np.ndarray, dtype) -> np.ndarray:
    return np.ascontiguousarray(v.reshape(-1, P).T.astype(dtype))


def _prep_weights(cfg, W1, b1, bn_gamma, bn_beta, bn_mean, bn_var, W_lin1, b_lin1,
                  p1, Wl, Wr, att, b_gat, p2, W_lin2, b_lin2, W_lin3, b_lin3):
    f32 = np.float32
    bn_a = (bn_gamma / np.sqrt(bn_var + 1e-5)).astype(f32)
    bn_b = (bn_beta - bn_mean * bn_a).astype(f32)
    W23 = (W_lin2 @ W_lin3).reshape(-1).astype(f32)
    Cc = np.array([cfg.k2 * float(b_lin2 @ W_lin3[:, 0]) + float(b_lin3[0])], dtype=f32)
    return {
        "W1": np.ascontiguousarray(W1, f32), "Wlin1": np.ascontiguousarray(W_lin1, f32),
        "Wl": np.ascontiguousarray(Wl, f32), "Wr": np.ascontiguousarray(Wr, f32),
        "b1": np.ascontiguousarray(b1, f32), "bn_a": bn_a, "bn_b": bn_b,
        "b_lin1": np.ascontiguousarray(b_lin1, f32),
        "p1": (np.asarray(p1) / np.linalg.norm(np.asarray(p1))).astype(f32),
        "att": np.ascontiguousarray(att, f32), "b_gat": np.ascontiguousarray(b_gat, f32),
        "p2": (np.asarray(p2) / np.linalg.norm(np.asarray(p2))).astype(f32),
        "W23": W23, "Cc": Cc,
    }


def _prep_core_edges(cfg: Cfg, src_core, dst_core):
    """src/dst core-local [ne]. Builds:
    - dlo/dhi histogram tables (graph-blocked order, graph-local dst bits)
    - srcw/dstw edge lists (core-local ids) window-packed so every CH-sized
      window has DISTINCT dst rows (sorted-by-dst position p -> window p%K),
      which makes each dma_scatter_add call free of same-row RMW races."""
    loops = np.arange(cfg.npg, dtype=np.int64)
    sl, dl, dll = [], [], []
    for g in range(cfg.ng):
        e = slice(g * cfg.eg, (g + 1) * cfg.eg)
        sl.append(src_core[e])
        sl.append(loops + g * cfg.npg)
        dl.append(dst_core[e])
        dl.append(loops + g * cfg.npg)
        dll.append(dst_core[e] - g * cfg.npg)
        dll.append(loops)
    src = np.concatenate(sl)
    dst = np.concatenate(dl)
    dst_gl = np.concatenate(dll)  # graph-local, graph-blocked (for histogram)

    K = cfg.ne2 // cfg.ch
    order = np.argsort(dst, kind="stable")
    perm = order.reshape(cfg.ch, K).T.reshape(-1)
    srcp, dstp = src[perm], dst[perm]
    dw = dstp.reshape(K, cfg.ch)
    for k in range(K):
        assert len(np.unique(dw[k])) == cfg.ch, "window has duplicate dst"
    return {
        "srcw": _wrap_idx(srcp),
        "dstw": _wrap_idx(dstp),
        "dlo": _pm_layout(dst_gl & 127, ml_dtypes.bfloat16),
        "dhi": _pm_layout(dst_gl >> 7, ml_dtypes.bfloat16),
    }


def build_bass(cfg: Cfg):
    from contextlib import ExitStack
    nc = bacc.Bacc("TRN2", target_bir_lowering=False, debug=False,
                   num_swdge_queues=4)
    with tile.TileContext(nc) as tc:
        with ExitStack() as ctx:
            build_core_program(ctx, tc, cfg)
    nc.compile()
    return nc


_CFG = Cfg()
_NC_CACHE = {}
TRACE = False
LAST_RESULT = None


def kernel(x, edge_index, batch, W1, b1, bn_gamma, bn_beta, bn_mean, bn_var,
           W_lin1, b_lin1, p1, Wl, Wr, att, b_gat, p2,
           W_lin2, b_lin2, W_lin3, b_lin3):
    cfg = _CFG
    n_cores = 8
    s_att = float(np.sum(np.asarray(att, dtype=np.float64)))
    assert abs(s_att) > 1e-6, "degenerate att sum; poison scheme needs |sum(att)|>0"
    cfg.psign = -1.0 if s_att > 0 else 1.0
    # poisoned edge: e ~= -slope * pb_mag * |s_att| = -40 (exp LUT safe, w ~ 0);
    # leaky slope seen by the poison is 0.2 for negative poison, 1.0 for positive
    slope = 0.2 if s_att > 0 else 1.0
    cfg.pb_mag = 40.0 / (slope * abs(s_att))
    weights = _prep_weights(cfg, W1, b1, bn_gamma, bn_beta, bn_mean, bn_var,
                            W_lin1, b_lin1, p1, Wl, Wr, att, b_gat, p2,
                            W_lin2, b_lin2, W_lin3, b_lin3)
    src_all = np.asarray(edge_index[0], dtype=np.int64)
    dst_all = np.asarray(edge_index[1], dtype=np.int64)
    x = np.asarray(x, dtype=np.float32)

    in_maps = []
    for c in range(n_cores):
        n0 = c * cfg.nn
        e0 = c * cfg.ne
        d = dict(weights)
        d.update(
            _prep_core_edges(
                cfg, src_all[e0 : e0 + cfg.ne] - n0, dst_all[e0 : e0 + cfg.ne] - n0
            )
        )
        d["xT"] = np.ascontiguousarray(x[n0 : n0 + cfg.nn].T, np.float32)
        in_maps.append(d)

    key = ("nc", cfg.psign, cfg.pb_mag)
    if key not in _NC_CACHE:
        _NC_CACHE[key] = build_bass(cfg)
    nc = _NC_CACHE[key]
    global LAST_RESULT
    res = run_bass_kernel_spmd(nc, in_maps, core_ids=list(range(n_cores)), trace=TRACE)
    LAST_RESULT = res
    outs = [np.asarray(res.results[c]["out"]).reshape(cfg.ng, 1) for c in range(n_cores)]
    return np.concatenate(outs, axis=0).astype(np.float32)



# revision 13
# speedup vs baseline: 2.1077x; 2.1077x over previous
"""Trainium2 Bass kernel for nn_AGNN_EFG (GCN -> TopK pool -> GATv2 -> TopK pool -> head).

Self-contained: shards the B=64 graphs across 8 NeuronCores (8 graphs/core),
runs one SPMD Bass program, gathers the [64, 1] head output on host.

v3 design (vs v2 baseline which used indirect DMA for all 5 edge passes):
- Edges (incl self loops) are sorted per graph by dst BLOCK (db = dst>>7),
  each db run padded to a fixed number spd of 128-edge chunks, so the
  chunk -> db map is static and identical across cores (SPMD-safe).
- Scatter-adds are ONE-HOT MATMULS on the (otherwise idle) PE engine:
  per chunk a [128e,128d] bf16 one-hot (built on DVE from dst low bits,
  pad slots use sentinel 255 -> all-zero rows) scatters messages into a
  per-graph PSUM accumulator. No dma_scatter_add anywhere.
- GAT's xr[dst] per edge is also a one-hot matmul ([128d,128e] orientation)
  reading xr blocks straight from SBUF. No dst-gather.
- Only 2 indirect passes remain (GpSimd was the baseline bottleneck at
  70% busy): gather u[src] for GCN and gather [xl|xr][src] for GAT.
- GCN u rows are split [u_hi|u_lo] bf16 (f32 accuracy after the one
  matmul per chunk against the 128-wide row).
- Degrees are host-precomputed index data (bincount of dst); rsqrt on
  device. Kills the on-device histogram phase.
"""

import sys

sys.path.insert(0, "/opt/trn_rl_repo")

from dataclasses import dataclass

import numpy as np
import ml_dtypes

import concourse.bass as bass
import concourse.mybir as mybir
import concourse.tile as tile
from concourse import bacc
from concourse.bass_utils import run_bass_kernel_spmd
from concourse.masks import make_identity

P = 128
F32 = mybir.dt.float32
BF16 = mybir.dt.bfloat16
I16 = mybir.dt.int16
U8 = mybir.dt.uint8
AF = mybir.ActivationFunctionType
OP = mybir.AluOpType
AX = mybir.AxisListType


@dataclass
class Cfg:
    ng: int = 8          # graphs per core
    npg: int = 2048      # nodes per graph
    hid: int = 64        # feature dim
    eg: int = 32768      # edges per graph (original, without self loops)
    spd: int = 19        # chunks (of 128 slots) per dst-block run; set at runtime
    ch: int = 1024       # gather window (slots per dma_gather call)
    n_bisect: int = 30   # bisection iterations for topk threshold
    psign: float = -1.0  # -sign(sum(att)): poison sign so poisoned e < 0
    pb_mag: float = 200.0  # poison magnitude; set so poisoned e ~ -40 (LUT-safe)

    @property
    def nn(self):
        return self.ng * self.npg

    @property
    def ne(self):
        return self.ng * self.eg

    @property
    def tj(self):
        return self.npg // P  # dst blocks per graph (16)

    @property
    def nt(self):
        return self.ng * self.tj  # 128

    @property
    def tch(self):
        return self.tj * self.spd  # chunks per graph

    @property
    def slots_g(self):
        return self.tch * P  # padded edge slots per graph

    @property
    def slots(self):
        return self.ng * self.slots_g

    @property
    def wpg(self):
        assert self.slots_g % self.ch == 0
        return self.slots_g // self.ch  # gather windows per graph

    @property
    def cpw(self):
        return self.ch // P  # chunks per window (8)

    @property
    def k1(self):
        return self.npg // 2

    @property
    def k2(self):
        return self.npg // 4


def build_core_program(ctx, tc, cfg: Cfg):
    nc = tc.nc
    NG, NPG, HID, NN = cfg.ng, cfg.npg, cfg.hid, cfg.nn
    NT, TJ, SPD = cfg.nt, cfg.tj, cfg.spd
    TCH, WPG, CPW, CH = cfg.tch, cfg.wpg, cfg.cpw, cfg.ch
    SRUN = SPD * P  # slots per db run
    assert HID == 64 and CPW == 8

    # ---- I/O ----
    xT = nc.dram_tensor("xT", [HID, NN], F32, kind="ExternalInput").ap()
    srcw = nc.dram_tensor("srcw", [P, cfg.slots // 16], I16, kind="ExternalInput").ap()
    dlo_pm = nc.dram_tensor("dlo_pm", [P, NG * TCH], BF16, kind="ExternalInput").ap()
    dlo_fr = nc.dram_tensor("dlo_fr", [NG * TJ * SRUN], BF16, kind="ExternalInput").ap()
    degT = nc.dram_tensor("degT", [P, NT], F32, kind="ExternalInput").ap()
    w_names = ["W1", "Wlin1", "Wl", "Wr"]
    Wd = {n: nc.dram_tensor(n, [HID, HID], F32, kind="ExternalInput").ap() for n in w_names}
    v_names = ["b1", "bn_a", "bn_b", "b_lin1", "p1", "att", "b_gat", "p2", "W23"]
    Vd = {n: nc.dram_tensor(n, [HID], F32, kind="ExternalInput").ap() for n in v_names}
    Cd = nc.dram_tensor("Cc", [1], F32, kind="ExternalInput").ap()
    out_d = nc.dram_tensor("out", [NG, 1], F32, kind="ExternalOutput").ap()

    # ---- DRAM scratch ----
    utab = nc.dram_tensor("utab", [NN, P], BF16).ap()      # rows [u_hi | u_lo]
    gtab = nc.dram_tensor("gtab", [NN, P], BF16).ap()      # rows [xl | xr] gated
    xlr_dram = nc.dram_tensor("xlr_dram", [P, NN], BF16).ap()
    sc_dram = nc.dram_tensor("sc_dram", [NN], F32).ap()

    cpool = ctx.enter_context(tc.tile_pool(name="consts", bufs=1))
    mpool = ctx.enter_context(tc.tile_pool(name="main", bufs=1))
    ppool = ctx.enter_context(tc.tile_pool(name="psum", bufs=1, space="PSUM"))

    # ---- constants ----
    ident = cpool.tile([P, P], F32)
    make_identity(nc, ident[:])
    ones128 = cpool.tile([P, P], F32)
    nc.gpsimd.memset(ones128[:], 1.0)
    nantile = cpool.tile([P, NT], F32)
    nc.gpsimd.memset(nantile[:], float("nan"))
    negbig = cpool.tile([P, NT], F32)
    nc.gpsimd.memset(negbig[:], -1e9)
    io16 = cpool.tile([P, P], I16)
    nc.gpsimd.iota(io16[:], pattern=[[1, P]], base=0, channel_multiplier=0)
    iota_bf = cpool.tile([P, P], BF16)
    nc.vector.tensor_copy(out=iota_bf[:], in_=io16[:])
    ioc16 = cpool.tile([P, 1], I16)
    nc.gpsimd.iota(ioc16[:], pattern=[[0, 1]], base=0, channel_multiplier=1)
    iota_col = cpool.tile([P, 1], F32)
    nc.vector.tensor_copy(out=iota_col[:], in_=ioc16[:])

    Ws = {}
    for n in w_names:
        t = cpool.tile([HID, HID], F32, tag=f"w_{n}")
        nc.sync.dma_start(out=t[:], in_=Wd[n][:])
        Ws[n] = t
    Vs = {}
    for n in v_names:
        t = cpool.tile([HID, 1], F32, tag=f"v_{n}")
        nc.sync.dma_start(out=t[:], in_=Vd[n][:, None])
        Vs[n] = t
    att_rep = cpool.tile([P, HID], BF16)
    nc.gpsimd.dma_start(out=att_rep[:], in_=Vd["att"][None, :].to_broadcast([P, HID]))
    p2_rep = cpool.tile([P, HID], F32)
    nc.sync.dma_start(out=p2_rep[:], in_=Vd["p2"][None, :].to_broadcast([P, HID]))
    bgat_rep = cpool.tile([P, HID], F32)
    nc.sync.dma_start(out=bgat_rep[:], in_=Vd["b_gat"][None, :].to_broadcast([P, HID]))
    Cc_sb = cpool.tile([NG, 1], F32)
    nc.sync.dma_start(out=Cc_sb[:], in_=Cd[None, :].to_broadcast([NG, 1]))

    # whole per-chunk dst-low-bit table (for one-hot builds in [e,d] orientation)
    dlo_sb = cpool.tile([P, NG * TCH], BF16)
    nc.sync.dma_start(out=dlo_sb[:], in_=dlo_pm[:])

    # ---- dinv from host degree counts ----
    dinv_t = mpool.tile([P, NT], F32, tag="dinv_t")
    sqd_t = mpool.tile([P, NT], F32, tag="sqd_t")
    ntmp = mpool.tile([P, NT], F32, tag="ntmp")

    def recip_newton(r_ap, x_ap, tmp_ap):
        nc.vector.tensor_tensor(out=tmp_ap, in0=x_ap, in1=r_ap, op=OP.mult)
        nc.vector.tensor_scalar(
            out=tmp_ap, in0=tmp_ap, scalar1=-1.0, scalar2=2.0, op0=OP.mult, op1=OP.add
        )
        nc.vector.tensor_tensor(out=r_ap, in0=r_ap, in1=tmp_ap, op=OP.mult)

    deg_sb = mpool.tile([P, NT], F32, tag="deg_sb")
    nc.sync.dma_start(out=deg_sb[:], in_=degT[:])
    nc.scalar.sqrt(out=sqd_t[:], in_=deg_sb[:])
    nc.vector.reciprocal(out=dinv_t[:], in_=sqd_t[:])
    recip_newton(dinv_t[:], sqd_t[:], ntmp[:])

    def idx_slice(pool, w_global, tag):
        t = pool.tile([P, CH // 16], I16, tag=tag)
        c0 = w_global * (CH // 16)
        nc.sync.dma_start(out=t[:], in_=srcw[:, c0 : c0 + CH // 16])
        return t

    from contextlib import ExitStack as _ES

    # ======== u-table: per graph, rows [dinv*xW1 as bf16 hi | lo] ========
    phase_u = _ES()
    s3p = phase_u.enter_context(tc.tile_pool(name="s3p", bufs=2, space="PSUM"))
    s3s = phase_u.enter_context(tc.tile_pool(name="s3s", bufs=2))
    for g in range(NG):
        xTg = s3s.tile([HID, NPG], F32, tag="xTg", name=f"xTg{g}")
        nc.sync.dma_start(out=xTg[:], in_=xT[:, g * NPG : (g + 1) * NPG])
        u_stage = s3s.tile([P, TJ, HID], F32, tag="ustage", name=f"ustage{g}")
        for jl in range(TJ):
            j = g * TJ + jl
            pm = s3p.tile([HID, P], F32, tag="xwps", name=f"xwps{g}_{jl}")
            nc.tensor.matmul(
                pm[:], lhsT=Ws["W1"][:], rhs=xTg[:, jl * P : (jl + 1) * P],
                start=True, stop=True,
            )
            xw_t = s3s.tile([HID, P], F32, tag="xwsb", name=f"xwsb{g}_{jl}")
            nc.scalar.copy(out=xw_t[:], in_=pm[:])
            pt = s3p.tile([P, HID], F32, tag="trps", name=f"trps{g}_{jl}")
            nc.tensor.transpose(out=pt[:], in_=xw_t[:], identity=ident[:HID, :HID])
            nc.vector.tensor_scalar(
                out=u_stage[:, jl, :], in0=pt[:], scalar1=dinv_t[:, j : j + 1],
                scalar2=None, op0=OP.mult,
            )
        uhl = s3s.tile([P, TJ, P], BF16, tag="uhl", name=f"uhl{g}")
        nc.vector.tensor_copy(out=uhl[:, :, 0:HID], in_=u_stage[:])
        uhi_f = s3s.tile([P, TJ, HID], F32, tag="uhif", name=f"uhif{g}")
        nc.vector.tensor_copy(out=uhi_f[:], in_=uhl[:, :, 0:HID])
        nc.vector.tensor_tensor(
            out=uhl[:, :, HID:P], in0=u_stage[:], in1=uhi_f[:], op=OP.subtract
        )
        nc.sync.dma_start(
            out=utab.rearrange("(j p) f -> p j f", p=P)[:, g * TJ : (g + 1) * TJ, :],
            in_=uhl[:],
        )
    phase_u.close()

    # ======== GCN edge pass (one-hot matmul scatter) + dense per graph ========
    xlr = mpool.tile([P, NN], BF16, tag="bigC2")
    S6C = 512
    phase_g = _ES()
    gep = phase_g.enter_context(tc.tile_pool(name="gep", bufs=6))
    ohp = phase_g.enter_context(tc.tile_pool(name="ohp", bufs=4))
    hps_pool = phase_g.enter_context(tc.tile_pool(name="hpsp", bufs=1, space="PSUM"))
    s5s = phase_g.enter_context(tc.tile_pool(name="s5s", bufs=2))
    s5p = phase_g.enter_context(tc.tile_pool(name="s5p", bufs=1, space="PSUM"))
    s6s = phase_g.enter_context(tc.tile_pool(name="s6s", bufs=2))
    s6p = phase_g.enter_context(tc.tile_pool(name="s6p", bufs=1, space="PSUM"))

    for g in range(NG):
        hps = [
            hps_pool.tile([P, 8, HID], F32, tag=f"hps{t}", name=f"hps{t}_{g}")
            for t in range(2)
        ]
        for w in range(WPG):
            wg = g * WPG + w
            ssl = idx_slice(gep, wg, "ssl")
            ub = gep.tile([P, CPW, P], BF16, tag="ub")
            nc.gpsimd.dma_gather(
                out_ap=ub[:], in_ap=utab[:], idxs_ap=ssl[:],
                num_idxs=CH, num_idxs_reg=CH, elem_size=P, queue_num=wg % 4,
            )
            oh8 = ohp.tile([P, CPW, P], BF16, tag="oh8")
            c0 = g * TCH + w * CPW
            nc.vector.tensor_tensor(
                out=oh8[:],
                in0=iota_bf[:, None, :].to_broadcast([P, CPW, P]),
                in1=dlo_sb[:, c0 : c0 + CPW, None].to_broadcast([P, CPW, P]),
                op=OP.is_equal,
            )
            for c in range(CPW):
                gc = w * CPW + c
                db, pos = gc // SPD, gc % SPD
                out_slc = hps[db // 8][:, db % 8, :]
                nc.tensor.matmul(
                    out_slc, lhsT=oh8[:, c, :], rhs=ub[:, c, 0:HID],
                    start=(pos == 0), stop=False,
                )
                nc.tensor.matmul(
                    out_slc, lhsT=oh8[:, c, :], rhs=ub[:, c, HID:P],
                    start=False, stop=(pos == SPD - 1),
                )

        # ---- finalize graph: h = dinv*(hi+lo); BN; leaky; lin1; scores; xl/xr
        hfm = s5s.tile([HID, NPG], F32, tag="hfm", name=f"hfm{g}")
        for jl in range(TJ):
            j = g * TJ + jl
            hs = s5s.tile([P, HID], F32, tag="hs", name=f"hs{g}_{jl}")
            nc.vector.tensor_scalar(
                out=hs[:], in0=hps[jl // 8][:, jl % 8, :], scalar1=dinv_t[:, j : j + 1],
                scalar2=None, op0=OP.mult,
            )
            pt = s5p.tile([HID, P], F32, tag="htr", name=f"htr{g}_{jl}")
            nc.tensor.transpose(out=pt[:], in_=hs[:], identity=ident[:])
            nc.scalar.copy(out=hfm[:, jl * P : (jl + 1) * P], in_=pt[:])
        nc.vector.tensor_scalar(
            out=hfm[:], in0=hfm[:], scalar1=Vs["b1"][:], scalar2=None, op0=OP.add
        )
        nc.vector.tensor_scalar(
            out=hfm[:], in0=hfm[:], scalar1=Vs["bn_a"][:], op0=OP.mult,
            scalar2=Vs["bn_b"][:], op1=OP.add,
        )
        nc.vector.scalar_tensor_tensor(
            out=hfm[:], in0=hfm[:], scalar=0.01, in1=hfm[:], op0=OP.mult, op1=OP.max,
        )
        for ol in range(0, NPG, S6C):
            o = g * NPG + ol
            pm = s6p.tile([HID, S6C], F32, tag="l1ps", name=f"l1ps{o}")
            nc.tensor.matmul(
                pm[:], lhsT=Ws["Wlin1"][:], rhs=hfm[:, ol : ol + S6C],
                start=True, stop=True,
            )
            hc = s6s.tile([HID, S6C], F32, tag="hc", name=f"hc{o}")
            nc.scalar.activation(
                out=hc[:], in_=pm[:], func=AF.Identity, bias=Vs["b_lin1"][:]
            )
            ps = s6p.tile([1, S6C], F32, tag="scps", name=f"scps{o}")
            nc.tensor.matmul(ps[:], lhsT=Vs["p1"][:], rhs=hc[:], start=True, stop=True)
            sc = s6s.tile([1, S6C], F32, tag="scsb", name=f"scsb{o}")
            nc.vector.tensor_copy(out=sc[:], in_=ps[:])
            nc.sync.dma_start(out=sc_dram[None, o : o + S6C], in_=sc[:])
            px = s6p.tile([P, S6C], F32, tag="xlrps", name=f"xlrps{o}")
            nc.tensor.matmul(px[:HID, :], lhsT=Ws["Wl"][:], rhs=hc[:], start=True, stop=True)
            nc.tensor.matmul(px[HID:, :], lhsT=Ws["Wr"][:], rhs=hc[:], start=True, stop=True)
            nc.scalar.copy(out=xlr[:, o : o + S6C], in_=px[:])

    phase_g.close()
    score1_t = mpool.tile([P, NT], F32, tag="score1_t")
    nc.sync.dma_start(out=score1_t[:], in_=sc_dram.rearrange("(j p) -> p j", p=P))

    # ---- bisection: per-graph threshold so that #(score > t) == target ----
    def bisect_threshold(score_t, target, tag):
        lo = mpool.tile([P, NG], F32, tag=f"lo_{tag}", name=f"lo_{tag}")
        hi = mpool.tile([P, NG], F32, tag=f"hi_{tag}", name=f"hi_{tag}")
        mid = mpool.tile([P, NG], F32, tag=f"mid_{tag}", name=f"mid_{tag}")
        cmp = mpool.tile([P, NT], F32, tag=f"cmp_{tag}", name=f"cmp_{tag}")
        cred = mpool.tile([P, NG], F32, tag=f"cred_{tag}", name=f"cred_{tag}")
        ge = mpool.tile([P, NG], U8, tag=f"ge_{tag}", name=f"ge_{tag}")
        lt = mpool.tile([P, NG], U8, tag=f"lt_{tag}", name=f"lt_{tag}")
        nc.gpsimd.memset(lo[:], -64.0)
        nc.gpsimd.memset(hi[:], 64.0)
        sc_g = score_t[:].rearrange("p (g t) -> p g t", g=NG)
        cmp_g = cmp[:].rearrange("p (g t) -> p g t", g=NG)
        for it in range(cfg.n_bisect):
            nc.vector.tensor_tensor(out=mid[:], in0=lo[:], in1=hi[:], op=OP.add)
            nc.vector.tensor_scalar(
                out=mid[:], in0=mid[:], scalar1=0.5, scalar2=None, op0=OP.mult
            )
            nc.vector.tensor_tensor(
                out=cmp_g, in0=sc_g,
                in1=mid[:, :, None].to_broadcast([P, NG, TJ]), op=OP.is_gt,
            )
            nc.vector.tensor_reduce(out=cred[:], in_=cmp_g, axis=AX.X, op=OP.add)
            cps = ppool.tile([P, NG], F32, tag="pp", name=f"cnt_{tag}_{it}")
            nc.tensor.matmul(cps[:], lhsT=ones128[:], rhs=cred[:], start=True, stop=True)
            nc.vector.tensor_scalar(
                out=ge[:], in0=cps[:], scalar1=float(target), scalar2=None, op0=OP.is_ge
            )
            nc.vector.tensor_scalar(
                out=lt[:], in0=cps[:], scalar1=float(target), scalar2=None, op0=OP.is_lt
            )
            nc.vector.copy_predicated(out=lo[:], mask=ge[:], data=mid[:])
            nc.vector.copy_predicated(out=hi[:], mask=lt[:], data=mid[:])
        return lo

    # ======== pool1 threshold; gate1 = tanh(score) kept else NaN ========
    t1 = bisect_threshold(score1_t, cfg.k1, "p1")
    kept1 = mpool.tile([P, NT], U8, tag="kept1")
    nc.vector.tensor_tensor(
        out=kept1[:].rearrange("p (g t) -> p g t", g=NG),
        in0=score1_t[:].rearrange("p (g t) -> p g t", g=NG),
        in1=t1[:, :, None].to_broadcast([P, NG, TJ]), op=OP.is_gt,
    )
    tanh1 = mpool.tile([P, NT], F32, tag="tanh1")
    nc.scalar.activation(out=tanh1[:], in_=score1_t[:], func=AF.Tanh)
    gate1 = mpool.tile([P, NT], F32, tag="gate1")
    nc.vector.tensor_copy(out=gate1[:], in_=nantile[:])
    nc.vector.copy_predicated(out=gate1[:], mask=kept1[:], data=tanh1[:])
    gate1z = mpool.tile([P, NT], F32, tag="gate1z")
    nc.vector.memset(gate1z[:], 0.0)
    nc.vector.copy_predicated(out=gate1z[:], mask=kept1[:], data=tanh1[:])
    padd = mpool.tile([P, NT], F32, tag="padd")
    pb = cfg.psign * cfg.pb_mag
    nc.vector.tensor_scalar(
        out=padd[:], in0=kept1[:], scalar1=-pb, scalar2=pb, op0=OP.mult, op1=OP.add
    )

    # ======== gtab = gate1 * transpose(xlr) (poisoned rows for dropped) ========
    nc.sync.dma_start(out=xlr_dram[:], in_=xlr[:])
    gstage = mpool.tile([P, NT, P], BF16, tag="bigC")
    nc.sync.dma_start_transpose(out=gstage[:], in_=xlr_dram[:])
    nc.vector.tensor_tensor(
        out=gstage[:], in0=gstage[:],
        in1=gate1z[:, :, None].to_broadcast([P, NT, P]), op=OP.mult,
    )
    nc.vector.tensor_tensor(
        out=gstage[:], in0=gstage[:],
        in1=padd[:, :, None].to_broadcast([P, NT, P]), op=OP.add,
    )
    nc.sync.dma_start(out=gtab.rearrange("(j p) f -> p j f", p=P), in_=gstage[:])

    # ======== GAT edge pass (gather xl[src]; xr[dst] + scatter via one-hots) ====
    h2t = mpool.tile([P, NT, HID], F32, tag="bigA")
    phase_a = _ES()
    gap = phase_a.enter_context(tc.tile_pool(name="gap", bufs=4))
    aohp = phase_a.enter_context(tc.tile_pool(name="aohp", bufs=4))
    runp = phase_a.enter_context(tc.tile_pool(name="runp", bufs=2))
    gatps = phase_a.enter_context(tc.tile_pool(name="gatps", bufs=1, space="PSUM"))
    mxrp = phase_a.enter_context(tc.tile_pool(name="mxrp", bufs=2, space="PSUM"))
    gfin = phase_a.enter_context(tc.tile_pool(name="gfin", bufs=1))

    for g in range(NG):
        nmps = [
            gatps.tile([P, 8, HID], F32, tag=f"nmps{t}", name=f"nmps{t}_{g}")
            for t in range(2)
        ]
        dnps = gatps.tile([P, TJ], F32, tag="dnps", name=f"dnps{g}")
        oh_de = {}
        for w in range(WPG):
            wg = g * WPG + w
            ssl = idx_slice(gap, wg, "assl")
            gx = gap.tile([P, CPW, P], BF16, tag="gx")
            nc.gpsimd.dma_gather(
                out_ap=gx[:], in_ap=gtab[:], idxs_ap=ssl[:],
                num_idxs=CH, num_idxs_reg=CH, elem_size=P, queue_num=wg % 4,
            )
            # one-hots [e,d] for scatter, batched over the window's 8 chunks
            oh8 = aohp.tile([P, CPW, P], BF16, tag="aoh8")
            c0 = g * TCH + w * CPW
            nc.vector.tensor_tensor(
                out=oh8[:],
                in0=iota_bf[:, None, :].to_broadcast([P, CPW, P]),
                in1=dlo_sb[:, c0 : c0 + CPW, None].to_broadcast([P, CPW, P]),
                op=OP.is_equal,
            )
            # xr[dst] per edge via [d,e]-orientation one-hot matmuls
            mxr = mxrp.tile([P, CPW, HID], F32, tag="mxr", name=f"mxr{wg}")
            for c in range(CPW):
                gc = w * CPW + c
                r, pos = gc // SPD, gc % SPD
                if pos == 0:
                    dlo_bc = runp.tile([P, SRUN], BF16, tag="dlobc", name=f"dlobc{g}_{r}")
                    o = (g * TJ + r) * SRUN
                    nc.sync.dma_start(
                        out=dlo_bc[:],
                        in_=dlo_fr[o : o + SRUN][None, :].to_broadcast([P, SRUN]),
                    )
                    ohr = runp.tile([P, SRUN], BF16, tag="ohde", name=f"ohde{g}_{r}")
                    nc.vector.tensor_scalar(
                        out=ohr[:], in0=dlo_bc[:], scalar1=iota_col[:],
                        scalar2=None, op0=OP.is_equal,
                    )
                    oh_de[r] = ohr
                nc.tensor.matmul(
                    mxr[:, c, :],
                    lhsT=oh_de[r][:, pos * P : (pos + 1) * P],
                    rhs=gstage[:, g * TJ + r, HID:P],
                    start=True, stop=True,
                )
            # e = att . leaky(xl_s + xr_d); w = exp(e); pay = w*xl
            z = gap.tile([P, CPW, HID], BF16, tag="z")
            nc.vector.tensor_tensor(
                out=z[:], in0=gx[:, :, 0:HID], in1=mxr[:], op=OP.add
            )
            nc.vector.scalar_tensor_tensor(
                out=z[:], in0=z[:], scalar=0.2, in1=z[:], op0=OP.mult, op1=OP.max,
            )
            nc.vector.tensor_tensor(
                out=z[:], in0=z[:],
                in1=att_rep[:, None, :].to_broadcast([P, CPW, HID]), op=OP.mult,
            )
            e8 = gap.tile([P, CPW], F32, tag="e8")
            nc.vector.tensor_reduce(out=e8[:], in_=z[:], axis=AX.X, op=OP.add)
            w8 = gap.tile([P, CPW], F32, tag="w8")
            nc.scalar.activation(out=w8[:], in_=e8[:], func=AF.Exp)
            w8b = gap.tile([P, CPW], BF16, tag="w8b")
            nc.vector.tensor_copy(out=w8b[:], in_=w8[:])
            pay = gap.tile([P, CPW, HID], BF16, tag="pay")
            nc.vector.tensor_tensor(
                out=pay[:], in0=gx[:, :, 0:HID],
                in1=w8b[:, :, None].to_broadcast([P, CPW, HID]), op=OP.mult,
            )
            for c in range(CPW):
                gc = w * CPW + c
                db, pos = gc // SPD, gc % SPD
                nc.tensor.matmul(
                    nmps[db // 8][:, db % 8, :],
                    lhsT=oh8[:, c, :], rhs=pay[:, c, :],
                    start=(pos == 0), stop=(pos == SPD - 1),
                )
                nc.tensor.matmul(
                    dnps[:, db : db + 1],
                    lhsT=oh8[:, c, :], rhs=w8b[:, c : c + 1],
                    start=(pos == 0), stop=(pos == SPD - 1),
                )

        # ---- finalize graph g: h2 = leaky(numer/denom + b_gat) ----
        numsb = gfin.tile([P, TJ, HID], F32, tag="numsb", name=f"numsb{g}")
        nc.scalar.copy(out=numsb[:, 0:8, :], in_=nmps[0][:])
        nc.scalar.copy(out=numsb[:, 8:TJ, :], in_=nmps[1][:])
        den = gfin.tile([P, TJ], F32, tag="den", name=f"den{g}")
        rec = gfin.tile([P, TJ], F32, tag="rec", name=f"rec{g}")
        dtmp = gfin.tile([P, TJ], F32, tag="dtmp", name=f"dtmp{g}")
        nc.vector.tensor_scalar(
            out=den[:], in0=dnps[:], scalar1=1e-16, scalar2=None, op0=OP.add
        )
        nc.vector.reciprocal(out=rec[:], in_=den[:])
        recip_newton(rec[:], den[:], dtmp[:])
        hslc = h2t[:, g * TJ : (g + 1) * TJ, :]
        nc.vector.tensor_tensor(
            out=hslc, in0=numsb[:],
            in1=rec[:, :, None].to_broadcast([P, TJ, HID]), op=OP.mult,
        )
        nc.vector.tensor_tensor(
            out=hslc, in0=hslc,
            in1=bgat_rep[:, None, :].to_broadcast([P, TJ, HID]), op=OP.add,
        )
        nc.vector.scalar_tensor_tensor(
            out=hslc, in0=hslc, scalar=0.01, in1=hslc, op0=OP.mult, op1=OP.max
        )

    phase_a.close()

    # ======== score2 (t-space), mask to kept1 ========
    score2_t = mpool.tile([P, NT], F32, tag="score2_t")
    s2prod = mpool.tile([P, NT, HID], F32, tag="bigC2", name="s2prod")
    nc.vector.tensor_tensor(
        out=s2prod[:], in0=h2t[:],
        in1=p2_rep[:, None, :].to_broadcast([P, NT, HID]), op=OP.mult,
    )
    nc.vector.tensor_reduce(out=score2_t[:], in_=s2prod[:], axis=AX.X, op=OP.add)
    kept1_t = mpool.tile([P, NT], U8, tag="kept1_t")
    nc.vector.tensor_tensor(out=kept1_t[:], in0=gate1[:], in1=gate1[:], op=OP.is_equal)
    sc2m = mpool.tile([P, NT], F32, tag="sc2m")
    nc.vector.tensor_copy(out=sc2m[:], in_=negbig[:])
    nc.vector.copy_predicated(out=sc2m[:], mask=kept1_t[:], data=score2_t[:])

    # ======== pool2 threshold + gate2 = tanh * mask ========
    t2 = bisect_threshold(sc2m, cfg.k2, "p2")
    gate2 = mpool.tile([P, NT], F32, tag="gate2")
    nc.vector.tensor_tensor(
        out=gate2[:].rearrange("p (g t) -> p g t", g=NG),
        in0=sc2m[:].rearrange("p (g t) -> p g t", g=NG),
        in1=t2[:, :, None].to_broadcast([P, NG, TJ]), op=OP.is_gt,
    )
    tanh2 = mpool.tile([P, NT], F32, tag="tanh2")
    sc2c = mpool.tile([P, NT], F32, tag="sc2c")
    nc.vector.tensor_scalar(
        out=sc2c[:], in0=sc2m[:], scalar1=-64.0, scalar2=None, op0=OP.max
    )
    nc.scalar.activation(out=tanh2[:], in_=sc2c[:], func=AF.Tanh)
    nc.vector.tensor_tensor(out=gate2[:], in0=gate2[:], in1=tanh2[:], op=OP.mult)

    # ======== T_g = sum_n gate2[n] * h2[n]; out = T @ W23 + C ========
    Tps = ppool.tile([P, NG], F32, tag="pp2")
    for j in range(NT):
        g = j // TJ
        nc.tensor.matmul(
            Tps[:HID, g : g + 1], lhsT=h2t[:, j, :], rhs=gate2[:, j : j + 1],
            start=(j % TJ == 0), stop=(j % TJ == TJ - 1),
        )
    Tsb = mpool.tile([HID, NG], F32, tag="Tsb")
    nc.scalar.copy(out=Tsb[:], in_=Tps[:HID, :])
    hps2 = ppool.tile([NG, 1], F32, tag="pp")
    nc.tensor.matmul(hps2[:], lhsT=Tsb[:], rhs=Vs["W23"][:], start=True, stop=True)
    outsb = mpool.tile([NG, 1], F32, tag="outsb")
    nc.vector.tensor_tensor(out=outsb[:], in0=hps2[:], in1=Cc_sb[:], op=OP.add)
    nc.sync.dma_start(out=out_d[:], in_=outsb[:])


# ================= host side =================

def _wrap_idx(ix: np.ndarray) -> np.ndarray:
    n = ix.shape[0]
    w = ix.reshape(n // 16, 16).T.astype(np.int16)
    return np.tile(w, (8, 1)).copy()


def _prep_weights(cfg, W1, b1, bn_gamma, bn_beta, bn_mean, bn_var, W_lin1, b_lin1,
                  p1, Wl, Wr, att, b_gat, p2, W_lin2, b_lin2, W_lin3, b_lin3):
    f32 = np.float32
    bn_a = (bn_gamma / np.sqrt(bn_var + 1e-5)).astype(f32)
    bn_b = (bn_beta - bn_mean * bn_a).astype(f32)
    W23 = (W_lin2 @ W_lin3).reshape(-1).astype(f32)
    Cc = np.array([cfg.k2 * float(b_lin2 @ W_lin3[:, 0]) + float(b_lin3[0])], dtype=f32)
    return {
        "W1": np.ascontiguousarray(W1, f32), "Wlin1": np.ascontiguousarray(W_lin1, f32),
        "Wl": np.ascontiguousarray(Wl, f32), "Wr": np.ascontiguousarray(Wr, f32),
        "b1": np.ascontiguousarray(b1, f32), "bn_a": bn_a, "bn_b": bn_b,
        "b_lin1": np.ascontiguousarray(b_lin1, f32),
        "p1": (np.asarray(p1) / np.linalg.norm(np.asarray(p1))).astype(f32),
        "att": np.ascontiguousarray(att, f32), "b_gat": np.ascontiguousarray(b_gat, f32),
        "p2": (np.asarray(p2) / np.linalg.norm(np.asarray(p2))).astype(f32),
        "W23": W23, "Cc": Cc,
    }


def _prep_core_edges(cfg: Cfg, src_core, dst_core):
    """src/dst core-local [ne]. Per graph: append self loops, bucket edges by
    dst block (db = dst>>7), pad each db run to spd*128 slots. Pad slots get
    src=0 (any valid row; killed by the one-hot) and dlo=255 (matches no
    iota value -> all-zero one-hot row/column)."""
    SPD, SRUN = cfg.spd, cfg.spd * P
    loops = np.arange(cfg.npg, dtype=np.int64)
    src_slots = np.zeros((cfg.ng, cfg.tj, SRUN), np.int64)
    dlo_slots = np.full((cfg.ng, cfg.tj, SRUN), 255, np.int64)
    deg = np.zeros((cfg.ng, cfg.npg), np.int64)
    for g in range(cfg.ng):
        e = slice(g * cfg.eg, (g + 1) * cfg.eg)
        s = np.concatenate([src_core[e] - g * cfg.npg, loops])
        d = np.concatenate([dst_core[e] - g * cfg.npg, loops])
        deg[g] = np.bincount(d, minlength=cfg.npg)
        db = d >> 7
        for b in range(cfg.tj):
            m = db == b
            cnt = int(m.sum())
            assert cnt <= SRUN, f"db run overflow: {cnt} > {SRUN}"
            src_slots[g, b, :cnt] = s[m] + g * cfg.npg
            dlo_slots[g, b, :cnt] = d[m] & 127
    stream_src = src_slots.reshape(-1)
    stream_dlo = dlo_slots.reshape(-1)
    deg_t = np.ascontiguousarray(
        deg.reshape(cfg.ng, cfg.tj, P).transpose(2, 0, 1).reshape(P, cfg.nt)
    ).astype(np.float32)
    bf16 = ml_dtypes.bfloat16
    return {
        "srcw": _wrap_idx(stream_src),
        "dlo_pm": np.ascontiguousarray(
            stream_dlo.reshape(-1, P).T.astype(bf16)
        ),
        "dlo_fr": np.ascontiguousarray(dlo_slots.reshape(-1).astype(bf16)),
        "degT": deg_t,
    }


def build_bass(cfg: Cfg):
    from contextlib import ExitStack
    nc = bacc.Bacc("TRN2", target_bir_lowering=False, debug=False,
                   num_swdge_queues=4)
    with tile.TileContext(nc) as tc:
        with ExitStack() as ctx:
            build_core_program(ctx, tc, cfg)
    nc.compile()
    return nc


_CFG = Cfg()
_NC_CACHE = {}
TRACE = False
LAST_RESULT = None


def kernel(x, edge_index, batch, W1, b1, bn_gamma, bn_beta, bn_mean, bn_var,
           W_lin1, b_lin1, p1, Wl, Wr, att, b_gat, p2,
           W_lin2, b_lin2, W_lin3, b_lin3):
    cfg = _CFG
    n_cores = 8
    s_att = float(np.sum(np.asarray(att, dtype=np.float64)))
    assert abs(s_att) > 1e-6, "degenerate att sum; poison scheme needs |sum(att)|>0"
    cfg.psign = -1.0 if s_att > 0 else 1.0
    slope = 0.2 if s_att > 0 else 1.0
    cfg.pb_mag = 40.0 / (slope * abs(s_att))
    weights = _prep_weights(cfg, W1, b1, bn_gamma, bn_beta, bn_mean, bn_var,
                            W_lin1, b_lin1, p1, Wl, Wr, att, b_gat, p2,
                            W_lin2, b_lin2, W_lin3, b_lin3)
    src_all = np.asarray(edge_index[0], dtype=np.int64)
    dst_all = np.asarray(edge_index[1], dtype=np.int64)
    x = np.asarray(x, dtype=np.float32)

    # choose the chunks-per-db-run capacity from the data (global max so the
    # single SPMD program fits every core)
    max_run = 0
    for c in range(n_cores):
        for g in range(cfg.ng):
            e0 = c * cfg.ne + g * cfg.eg
            d = dst_all[e0 : e0 + cfg.eg] - (c * cfg.nn + g * cfg.npg)
            cnts = np.bincount(d >> 7, minlength=cfg.tj) + P  # + self loops
            max_run = max(max_run, int(cnts.max()))
    cfg.spd = (max_run + P - 1) // P
    # windows of ch slots must tile a graph's slot range exactly
    while (cfg.tj * cfg.spd * P) % cfg.ch != 0:
        cfg.spd += 1

    in_maps = []
    for c in range(n_cores):
        n0 = c * cfg.nn
        e0 = c * cfg.ne
        d = dict(weights)
        d.update(
            _prep_core_edges(
                cfg, src_all[e0 : e0 + cfg.ne] - n0, dst_all[e0 : e0 + cfg.ne] - n0
            )
        )
        d["xT"] = np.ascontiguousarray(x[n0 : n0 + cfg.nn].T, np.float32)
        in_maps.append(d)

    key = ("nc", cfg.spd, cfg.psign, cfg.pb_mag)
    if key not in _NC_CACHE:
        _NC_CACHE[key] = build_bass(cfg)
    nc = _NC_CACHE[key]
    global LAST_RESULT
    res = run_bass_kernel_spmd(nc, in_maps, core_ids=list(range(n_cores)), trace=TRACE)
    LAST_RESULT = res
    outs = [np.asarray(res.results[c]["out"]).reshape(cfg.ng, 1) for c in range(n_cores)]
    return np.concatenate(outs, axis=0).astype(np.float32)


# revision 19
# speedup vs baseline: 2.1946x; 1.0412x over previous
"""Trainium2 Bass kernel for nn_AGNN_EFG (GCN -> TopK pool -> GATv2 -> TopK pool -> head).

Self-contained: shards the B=64 graphs across 8 NeuronCores (8 graphs/core),
runs one SPMD Bass program, gathers the [64, 1] head output on host.

v3 design (vs v2 baseline which used indirect DMA for all 5 edge passes):
- Edges (incl self loops) are sorted per graph by dst BLOCK (db = dst>>7),
  each db run padded to a fixed number spd of 128-edge chunks, so the
  chunk -> db map is static and identical across cores (SPMD-safe).
- Scatter-adds are ONE-HOT MATMULS on the (otherwise idle) PE engine:
  per chunk a [128e,128d] bf16 one-hot (built on DVE from dst low bits,
  pad slots use sentinel 255 -> all-zero rows) scatters messages into a
  per-graph PSUM accumulator. No dma_scatter_add anywhere.
- GAT's xr[dst] per edge is also a one-hot matmul ([128d,128e] orientation)
  reading xr blocks straight from SBUF. No dst-gather.
- Only 2 indirect passes remain (GpSimd was the baseline bottleneck at
  70% busy): gather u[src] for GCN and gather [xl|xr][src] for GAT.
- GCN u rows are split [u_hi|u_lo] bf16 (f32 accuracy after the one
  matmul per chunk against the 128-wide row).
- Degrees are host-precomputed index data (bincount of dst); rsqrt on
  device. Kills the on-device histogram phase.
"""

import sys

sys.path.insert(0, "/opt/trn_rl_repo")

from dataclasses import dataclass

import numpy as np
import ml_dtypes

import concourse.bass as bass
import concourse.mybir as mybir
import concourse.tile as tile
from concourse import bacc
from concourse.bass_utils import run_bass_kernel_spmd
from concourse.masks import make_identity

P = 128
F32 = mybir.dt.float32
BF16 = mybir.dt.bfloat16
I16 = mybir.dt.int16
U8 = mybir.dt.uint8
AF = mybir.ActivationFunctionType
OP = mybir.AluOpType
AX = mybir.AxisListType


@dataclass
class Cfg:
    ng: int = 8          # graphs per core
    npg: int = 2048      # nodes per graph
    hid: int = 64        # feature dim
    eg: int = 32768      # edges per graph (original, without self loops)
    spd: int = 19        # chunks (of 128 slots) per dst-block run; set at runtime
    ch: int = 1024       # gather window (slots per dma_gather call; >=2048
                         # overflows the SWDGE descriptor ring and faults)
    n_bisect: int = 30   # bisection iterations for topk threshold
    psign: float = -1.0  # -sign(sum(att)): poison sign so poisoned e < 0
    pb_mag: float = 200.0  # poison magnitude; set so poisoned e ~ -40 (LUT-safe)

    @property
    def nn(self):
        return self.ng * self.npg

    @property
    def ne(self):
        return self.ng * self.eg

    @property
    def tj(self):
        return self.npg // P  # dst blocks per graph (16)

    @property
    def nt(self):
        return self.ng * self.tj  # 128

    @property
    def tch(self):
        return self.tj * self.spd  # chunks per graph

    @property
    def slots_g(self):
        return self.tch * P  # padded edge slots per graph

    @property
    def slots(self):
        return self.ng * self.slots_g

    @property
    def wpg(self):
        assert self.slots_g % self.ch == 0
        return self.slots_g // self.ch  # gather windows per graph

    @property
    def cpw(self):
        return self.ch // P  # chunks per window (8)

    @property
    def k1(self):
        return self.npg // 2

    @property
    def k2(self):
        return self.npg // 4


def build_core_program(ctx, tc, cfg: Cfg):
    nc = tc.nc
    NG, NPG, HID, NN = cfg.ng, cfg.npg, cfg.hid, cfg.nn
    NT, TJ, SPD = cfg.nt, cfg.tj, cfg.spd
    TCH, WPG, CPW, CH = cfg.tch, cfg.wpg, cfg.cpw, cfg.ch
    SRUN = SPD * P  # slots per db run
    assert HID == 64 and CPW % 8 == 0

    # ---- I/O ----
    xT = nc.dram_tensor("xT", [HID, NN], F32, kind="ExternalInput").ap()
    srcw = nc.dram_tensor("srcw", [P, cfg.slots // 16], I16, kind="ExternalInput").ap()
    dlo_pm = nc.dram_tensor("dlo_pm", [P, NG * TCH], BF16, kind="ExternalInput").ap()
    dlo_fr = nc.dram_tensor("dlo_fr", [NG * TJ * SRUN], BF16, kind="ExternalInput").ap()
    degT = nc.dram_tensor("degT", [P, NT], F32, kind="ExternalInput").ap()
    w_names = ["W1", "Wlin1", "Wl", "Wr"]
    Wd = {n: nc.dram_tensor(n, [HID, HID], F32, kind="ExternalInput").ap() for n in w_names}
    v_names = ["b1", "bn_a", "bn_b", "b_lin1", "p1", "att", "b_gat", "p2", "W23"]
    Vd = {n: nc.dram_tensor(n, [HID], F32, kind="ExternalInput").ap() for n in v_names}
    Cd = nc.dram_tensor("Cc", [1], F32, kind="ExternalInput").ap()
    out_d = nc.dram_tensor("out", [NG, 1], F32, kind="ExternalOutput").ap()

    # ---- DRAM scratch ----
    utab = nc.dram_tensor("utab", [NN, P], BF16).ap()      # rows [u_hi | u_lo]
    gtab = nc.dram_tensor("gtab", [NN, P], BF16).ap()      # rows [xl | xr] gated
    xlr_dram = nc.dram_tensor("xlr_dram", [P, NN], BF16).ap()
    sc_dram = nc.dram_tensor("sc_dram", [NN], F32).ap()

    cpool = ctx.enter_context(tc.tile_pool(name="consts", bufs=1))
    mpool = ctx.enter_context(tc.tile_pool(name="main", bufs=1))
    ppool = ctx.enter_context(tc.tile_pool(name="psum", bufs=1, space="PSUM"))

    # ---- constants ----
    ident = cpool.tile([P, P], F32)
    make_identity(nc, ident[:])
    ones128 = cpool.tile([P, P], F32)
    nc.gpsimd.memset(ones128[:], 1.0)
    nantile = cpool.tile([P, NT], F32)
    nc.gpsimd.memset(nantile[:], float("nan"))
    negbig = cpool.tile([P, NT], F32)
    nc.gpsimd.memset(negbig[:], -1e9)
    io16 = cpool.tile([P, P], I16)
    nc.gpsimd.iota(io16[:], pattern=[[1, P]], base=0, channel_multiplier=0)
    iota_bf = cpool.tile([P, P], BF16)
    nc.vector.tensor_copy(out=iota_bf[:], in_=io16[:])
    ioc16 = cpool.tile([P, 1], I16)
    nc.gpsimd.iota(ioc16[:], pattern=[[0, 1]], base=0, channel_multiplier=1)
    iota_col = cpool.tile([P, 1], F32)
    nc.vector.tensor_copy(out=iota_col[:], in_=ioc16[:])
    niota_col = cpool.tile([P, 1], F32)
    nc.vector.tensor_scalar(
        out=niota_col[:], in0=iota_col[:], scalar1=-1.0, scalar2=None, op0=OP.mult
    )

    Ws = {}
    for n in w_names:
        t = cpool.tile([HID, HID], F32, tag=f"w_{n}")
        nc.sync.dma_start(out=t[:], in_=Wd[n][:])
        Ws[n] = t
    Vs = {}
    for n in v_names:
        t = cpool.tile([HID, 1], F32, tag=f"v_{n}")
        nc.sync.dma_start(out=t[:], in_=Vd[n][:, None])
        Vs[n] = t
    att_rep = cpool.tile([P, HID], BF16)
    nc.gpsimd.dma_start(out=att_rep[:], in_=Vd["att"][None, :].to_broadcast([P, HID]))
    p2_rep = cpool.tile([P, HID], F32)
    nc.sync.dma_start(out=p2_rep[:], in_=Vd["p2"][None, :].to_broadcast([P, HID]))
    bgat_rep = cpool.tile([P, HID], F32)
    nc.sync.dma_start(out=bgat_rep[:], in_=Vd["b_gat"][None, :].to_broadcast([P, HID]))
    Cc_sb = cpool.tile([NG, 1], F32)
    nc.sync.dma_start(out=Cc_sb[:], in_=Cd[None, :].to_broadcast([NG, 1]))

    # whole per-chunk dst-low-bit table (for one-hot builds in [e,d] orientation)
    dlo_sb = cpool.tile([P, NG * TCH], BF16)
    nc.sync.dma_start(out=dlo_sb[:], in_=dlo_pm[:])

    # ---- dinv from host degree counts ----
    dinv_t = mpool.tile([P, NT], F32, tag="dinv_t")
    sqd_t = mpool.tile([P, NT], F32, tag="sqd_t")
    ntmp = mpool.tile([P, NT], F32, tag="ntmp")

    def recip_newton(r_ap, x_ap, tmp_ap):
        nc.vector.tensor_tensor(out=tmp_ap, in0=x_ap, in1=r_ap, op=OP.mult)
        nc.vector.tensor_scalar(
            out=tmp_ap, in0=tmp_ap, scalar1=-1.0, scalar2=2.0, op0=OP.mult, op1=OP.add
        )
        nc.vector.tensor_tensor(out=r_ap, in0=r_ap, in1=tmp_ap, op=OP.mult)

    deg_sb = mpool.tile([P, NT], F32, tag="deg_sb")
    nc.sync.dma_start(out=deg_sb[:], in_=degT[:])
    nc.scalar.sqrt(out=sqd_t[:], in_=deg_sb[:])
    nc.vector.reciprocal(out=dinv_t[:], in_=sqd_t[:])
    recip_newton(dinv_t[:], sqd_t[:], ntmp[:])

    def idx_slice(pool, w_global, tag):
        t = pool.tile([P, CH // 16], I16, tag=tag)
        c0 = w_global * (CH // 16)
        nc.sync.dma_start(out=t[:], in_=srcw[:, c0 : c0 + CH // 16])
        return t

    from contextlib import ExitStack as _ES

    # ======== u-table: per graph, rows [dinv*xW1 as bf16 hi | lo] ========
    phase_u = _ES()
    s3p = phase_u.enter_context(tc.tile_pool(name="s3p", bufs=2, space="PSUM"))
    s3s = phase_u.enter_context(tc.tile_pool(name="s3s", bufs=2))
    for g in range(NG):
        xTg = s3s.tile([HID, NPG], F32, tag="xTg", name=f"xTg{g}")
        nc.sync.dma_start(out=xTg[:], in_=xT[:, g * NPG : (g + 1) * NPG])
        u_stage = s3s.tile([P, TJ, HID], F32, tag="ustage", name=f"ustage{g}")
        for jl in range(TJ):
            j = g * TJ + jl
            pm = s3p.tile([HID, P], F32, tag="xwps", name=f"xwps{g}_{jl}")
            nc.tensor.matmul(
                pm[:], lhsT=Ws["W1"][:], rhs=xTg[:, jl * P : (jl + 1) * P],
                start=True, stop=True,
            )
            xw_t = s3s.tile([HID, P], F32, tag="xwsb", name=f"xwsb{g}_{jl}")
            nc.scalar.copy(out=xw_t[:], in_=pm[:])
            pt = s3p.tile([P, HID], F32, tag="trps", name=f"trps{g}_{jl}")
            nc.tensor.transpose(out=pt[:], in_=xw_t[:], identity=ident[:HID, :HID])
            nc.vector.tensor_scalar(
                out=u_stage[:, jl, :], in0=pt[:], scalar1=dinv_t[:, j : j + 1],
                scalar2=None, op0=OP.mult,
            )
        uhl = s3s.tile([P, TJ, P], BF16, tag="uhl", name=f"uhl{g}")
        nc.vector.tensor_copy(out=uhl[:, :, 0:HID], in_=u_stage[:])
        uhi_f = s3s.tile([P, TJ, HID], F32, tag="uhif", name=f"uhif{g}")
        nc.vector.tensor_copy(out=uhi_f[:], in_=uhl[:, :, 0:HID])
        nc.vector.tensor_tensor(
            out=uhl[:, :, HID:P], in0=u_stage[:], in1=uhi_f[:], op=OP.subtract
        )
        nc.sync.dma_start(
            out=utab.rearrange("(j p) f -> p j f", p=P)[:, g * TJ : (g + 1) * TJ, :],
            in_=uhl[:],
        )
    phase_u.close()

    # ======== GCN edge pass (one-hot matmul scatter) + dense per graph ========
    xlr = mpool.tile([P, NN], BF16, tag="bigC2")
    S6C = 512
    phase_g = _ES()
    gep = phase_g.enter_context(tc.tile_pool(name="gep", bufs=6))
    ohp = phase_g.enter_context(tc.tile_pool(name="ohp", bufs=4))
    hps_pool = phase_g.enter_context(tc.tile_pool(name="hpsp", bufs=1, space="PSUM"))
    s5s = phase_g.enter_context(tc.tile_pool(name="s5s", bufs=2))
    s5p = phase_g.enter_context(tc.tile_pool(name="s5p", bufs=1, space="PSUM"))
    s6s = phase_g.enter_context(tc.tile_pool(name="s6s", bufs=2))
    s6p = phase_g.enter_context(tc.tile_pool(name="s6p", bufs=1, space="PSUM"))

    for g in range(NG):
        hps = [
            hps_pool.tile([P, 8, HID], F32, tag=f"hps{t}", name=f"hps{t}_{g}")
            for t in range(2)
        ]
        for w in range(WPG):
            wg = g * WPG + w
            ssl = idx_slice(gep, wg, "ssl")
            ub = gep.tile([P, CPW, P], BF16, tag="ub")
            nc.gpsimd.dma_gather(
                out_ap=ub[:], in_ap=utab[:], idxs_ap=ssl[:],
                num_idxs=CH, num_idxs_reg=CH, elem_size=P, queue_num=wg % 4,
            )
            for b in range(CPW // 8):
                oh8 = ohp.tile([P, 8, P], BF16, tag="oh8")
                c0 = g * TCH + w * CPW + b * 8
                nc.vector.tensor_tensor(
                    out=oh8[:],
                    in0=iota_bf[:, None, :].to_broadcast([P, 8, P]),
                    in1=dlo_sb[:, c0 : c0 + 8, None].to_broadcast([P, 8, P]),
                    op=OP.is_equal,
                )
                for cl in range(8):
                    c = b * 8 + cl
                    gc = w * CPW + c
                    db, pos = gc // SPD, gc % SPD
                    out_slc = hps[db // 8][:, db % 8, :]
                    nc.tensor.matmul(
                        out_slc, lhsT=oh8[:, cl, :], rhs=ub[:, c, 0:HID],
                        start=(pos == 0), stop=False,
                    )
                    nc.tensor.matmul(
                        out_slc, lhsT=oh8[:, cl, :], rhs=ub[:, c, HID:P],
                        start=False, stop=(pos == SPD - 1),
                    )

        # ---- finalize graph: h = dinv*(hi+lo); BN; leaky; lin1; scores; xl/xr
        hfm = s5s.tile([HID, NPG], F32, tag="hfm", name=f"hfm{g}")
        for jl in range(TJ):
            j = g * TJ + jl
            hs = s5s.tile([P, HID], F32, tag="hs", name=f"hs{g}_{jl}")
            nc.vector.tensor_scalar(
                out=hs[:], in0=hps[jl // 8][:, jl % 8, :], scalar1=dinv_t[:, j : j + 1],
                scalar2=None, op0=OP.mult,
            )
            pt = s5p.tile([HID, P], F32, tag="htr", name=f"htr{g}_{jl}")
            nc.tensor.transpose(out=pt[:], in_=hs[:], identity=ident[:])
            nc.scalar.copy(out=hfm[:, jl * P : (jl + 1) * P], in_=pt[:])
        nc.vector.tensor_scalar(
            out=hfm[:], in0=hfm[:], scalar1=Vs["b1"][:], scalar2=None, op0=OP.add
        )
        nc.vector.tensor_scalar(
            out=hfm[:], in0=hfm[:], scalar1=Vs["bn_a"][:], op0=OP.mult,
            scalar2=Vs["bn_b"][:], op1=OP.add,
        )
        nc.vector.scalar_tensor_tensor(
            out=hfm[:], in0=hfm[:], scalar=0.01, in1=hfm[:], op0=OP.mult, op1=OP.max,
        )
        for ol in range(0, NPG, S6C):
            o = g * NPG + ol
            pm = s6p.tile([HID, S6C], F32, tag="l1ps", name=f"l1ps{o}")
            nc.tensor.matmul(
                pm[:], lhsT=Ws["Wlin1"][:], rhs=hfm[:, ol : ol + S6C],
                start=True, stop=True,
            )
            hc = s6s.tile([HID, S6C], F32, tag="hc", name=f"hc{o}")
            nc.scalar.activation(
                out=hc[:], in_=pm[:], func=AF.Identity, bias=Vs["b_lin1"][:]
            )
            ps = s6p.tile([1, S6C], F32, tag="scps", name=f"scps{o}")
            nc.tensor.matmul(ps[:], lhsT=Vs["p1"][:], rhs=hc[:], start=True, stop=True)
            sc = s6s.tile([1, S6C], F32, tag="scsb", name=f"scsb{o}")
            nc.vector.tensor_copy(out=sc[:], in_=ps[:])
            nc.sync.dma_start(out=sc_dram[None, o : o + S6C], in_=sc[:])
            px = s6p.tile([P, S6C], F32, tag="xlrps", name=f"xlrps{o}")
            nc.tensor.matmul(px[:HID, :], lhsT=Ws["Wl"][:], rhs=hc[:], start=True, stop=True)
            nc.tensor.matmul(px[HID:, :], lhsT=Ws["Wr"][:], rhs=hc[:], start=True, stop=True)
            nc.scalar.copy(out=xlr[:, o : o + S6C], in_=px[:])

    phase_g.close()
    score1_t = mpool.tile([P, NT], F32, tag="score1_t")
    nc.sync.dma_start(out=score1_t[:], in_=sc_dram.rearrange("(j p) -> p j", p=P))

    # ---- bisection: per-graph threshold so that #(score > t) == target ----
    def bisect_threshold(score_t, target, tag):
        lo = mpool.tile([P, NG], F32, tag=f"lo_{tag}", name=f"lo_{tag}")
        hi = mpool.tile([P, NG], F32, tag=f"hi_{tag}", name=f"hi_{tag}")
        mid = mpool.tile([P, NG], F32, tag=f"mid_{tag}", name=f"mid_{tag}")
        cmp = mpool.tile([P, NT], F32, tag=f"cmp_{tag}", name=f"cmp_{tag}")
        cred = mpool.tile([P, NG], F32, tag=f"cred_{tag}", name=f"cred_{tag}")
        ge = mpool.tile([P, NG], U8, tag=f"ge_{tag}", name=f"ge_{tag}")
        lt = mpool.tile([P, NG], U8, tag=f"lt_{tag}", name=f"lt_{tag}")
        nc.gpsimd.memset(lo[:], -64.0)
        nc.gpsimd.memset(hi[:], 64.0)
        sc_g = score_t[:].rearrange("p (g t) -> p g t", g=NG)
        cmp_g = cmp[:].rearrange("p (g t) -> p g t", g=NG)
        for it in range(cfg.n_bisect):
            nc.vector.tensor_tensor(out=mid[:], in0=lo[:], in1=hi[:], op=OP.add)
            nc.vector.tensor_scalar(
                out=mid[:], in0=mid[:], scalar1=0.5, scalar2=None, op0=OP.mult
            )
            nc.vector.tensor_tensor(
                out=cmp_g, in0=sc_g,
                in1=mid[:, :, None].to_broadcast([P, NG, TJ]), op=OP.is_gt,
            )
            nc.vector.tensor_reduce(out=cred[:], in_=cmp_g, axis=AX.X, op=OP.add)
            cps = ppool.tile([P, NG], F32, tag="pp", name=f"cnt_{tag}_{it}")
            nc.tensor.matmul(cps[:], lhsT=ones128[:], rhs=cred[:], start=True, stop=True)
            nc.vector.tensor_scalar(
                out=ge[:], in0=cps[:], scalar1=float(target), scalar2=None, op0=OP.is_ge
            )
            nc.vector.tensor_scalar(
                out=lt[:], in0=cps[:], scalar1=float(target), scalar2=None, op0=OP.is_lt
            )
            nc.vector.copy_predicated(out=lo[:], mask=ge[:], data=mid[:])
            nc.vector.copy_predicated(out=hi[:], mask=lt[:], data=mid[:])
        return lo

    # ======== pool1 threshold; gate1 = tanh(score) kept else NaN ========
    t1 = bisect_threshold(score1_t, cfg.k1, "p1")
    kept1 = mpool.tile([P, NT], U8, tag="kept1")
    nc.vector.tensor_tensor(
        out=kept1[:].rearrange("p (g t) -> p g t", g=NG),
        in0=score1_t[:].rearrange("p (g t) -> p g t", g=NG),
        in1=t1[:, :, None].to_broadcast([P, NG, TJ]), op=OP.is_gt,
    )
    tanh1 = mpool.tile([P, NT], F32, tag="tanh1")
    nc.scalar.activation(out=tanh1[:], in_=score1_t[:], func=AF.Tanh)
    gate1 = mpool.tile([P, NT], F32, tag="gate1")
    nc.vector.tensor_copy(out=gate1[:], in_=nantile[:])
    nc.vector.copy_predicated(out=gate1[:], mask=kept1[:], data=tanh1[:])
    gate1z = mpool.tile([P, NT], F32, tag="gate1z")
    nc.vector.memset(gate1z[:], 0.0)
    nc.vector.copy_predicated(out=gate1z[:], mask=kept1[:], data=tanh1[:])
    padd = mpool.tile([P, NT], F32, tag="padd")
    pb = cfg.psign * cfg.pb_mag
    nc.vector.tensor_scalar(
        out=padd[:], in0=kept1[:], scalar1=-pb, scalar2=pb, op0=OP.mult, op1=OP.add
    )

    # ======== gtab = gate1 * transpose(xlr) (poisoned rows for dropped) ========
    nc.sync.dma_start(out=xlr_dram[:], in_=xlr[:])
    gstage = mpool.tile([P, NT, P], BF16, tag="bigC")
    nc.sync.dma_start_transpose(out=gstage[:], in_=xlr_dram[:])
    nc.vector.tensor_tensor(
        out=gstage[:], in0=gstage[:],
        in1=gate1z[:, :, None].to_broadcast([P, NT, P]), op=OP.mult,
    )
    nc.vector.tensor_tensor(
        out=gstage[:], in0=gstage[:],
        in1=padd[:, :, None].to_broadcast([P, NT, P]), op=OP.add,
    )
    nc.sync.dma_start(out=gtab.rearrange("(j p) f -> p j f", p=P), in_=gstage[:])

    # ======== GAT edge pass (gather xl[src]; xr[dst] + scatter via one-hots) ====
    h2t = mpool.tile([P, NT, HID], F32, tag="bigA")
    phase_a = _ES()
    gap = phase_a.enter_context(tc.tile_pool(name="gap", bufs=4))
    aohp = phase_a.enter_context(tc.tile_pool(name="aohp", bufs=4))
    runp = phase_a.enter_context(tc.tile_pool(name="runp", bufs=2))
    gatps = phase_a.enter_context(tc.tile_pool(name="gatps", bufs=1, space="PSUM"))
    mxrp = phase_a.enter_context(tc.tile_pool(name="mxrp", bufs=2, space="PSUM"))
    gfin = phase_a.enter_context(tc.tile_pool(name="gfin", bufs=1))

    for g in range(NG):
        nmps = [
            gatps.tile([P, 8, HID], F32, tag=f"nmps{t}", name=f"nmps{t}_{g}")
            for t in range(2)
        ]
        dnps = gatps.tile([P, TJ], F32, tag="dnps", name=f"dnps{g}")
        oh_de = {}
        for w in range(WPG):
            wg = g * WPG + w
            ssl = idx_slice(gap, wg, "assl")
            gx = gap.tile([P, CPW, P], BF16, tag="gx")
            nc.gpsimd.dma_gather(
                out_ap=gx[:], in_ap=gtab[:], idxs_ap=ssl[:],
                num_idxs=CH, num_idxs_reg=CH, elem_size=P, queue_num=wg % 4,
            )
            for b in range(CPW // 8):
                # one-hots [e,d] for scatter, batched over 8 chunks
                oh8 = aohp.tile([P, 8, P], BF16, tag="aoh8")
                c0 = g * TCH + w * CPW + b * 8
                nc.vector.tensor_tensor(
                    out=oh8[:],
                    in0=iota_bf[:, None, :].to_broadcast([P, 8, P]),
                    in1=dlo_sb[:, c0 : c0 + 8, None].to_broadcast([P, 8, P]),
                    op=OP.is_equal,
                )
                # xr[dst] per edge via [d,e]-orientation one-hot matmuls
                mxr = mxrp.tile([P, 8, HID], F32, tag="mxr", name=f"mxr{wg}_{b}")
                for cl in range(8):
                    c = b * 8 + cl
                    gc = w * CPW + c
                    r, pos = gc // SPD, gc % SPD
                    if pos == 0:
                        dlo_bc = runp.tile(
                            [P, SRUN], BF16, tag="dlobc", name=f"dlobc{g}_{r}"
                        )
                        o = (g * TJ + r) * SRUN
                        nc.sync.dma_start(
                            out=dlo_bc[:],
                            in_=dlo_fr[o : o + SRUN][None, :].to_broadcast([P, SRUN]),
                        )
                        # one-hot on the Scalar engine: relu(1 - (dlo - d)^2)
                        ohsq = runp.tile([P, SRUN], BF16, tag="ohsq", name=f"ohsq{g}_{r}")
                        nc.scalar.activation(
                            out=ohsq[:], in_=dlo_bc[:], func=AF.Square,
                            bias=niota_col[:],
                        )
                        ohr = runp.tile([P, SRUN], BF16, tag="ohde", name=f"ohde{g}_{r}")
                        nc.scalar.activation(
                            out=ohr[:], in_=ohsq[:], func=AF.Relu,
                            bias=1.0, scale=-1.0,
                        )
                        oh_de[r] = ohr
                    nc.tensor.matmul(
                        mxr[:, cl, :],
                        lhsT=oh_de[r][:, pos * P : (pos + 1) * P],
                        rhs=gstage[:, g * TJ + r, HID:P],
                        start=True, stop=True,
                    )
                # e = att . leaky(xl_s + xr_d); w = exp(e); pay = w*xl
                gxs = gx[:, b * 8 : (b + 1) * 8, :]
                z = gap.tile([P, 8, HID], BF16, tag="z")
                nc.vector.tensor_tensor(
                    out=z[:], in0=gxs[:, :, 0:HID], in1=mxr[:], op=OP.add
                )
                nc.vector.scalar_tensor_tensor(
                    out=z[:], in0=z[:], scalar=0.2, in1=z[:], op0=OP.mult, op1=OP.max,
                )
                nc.vector.tensor_tensor(
                    out=z[:], in0=z[:],
                    in1=att_rep[:, None, :].to_broadcast([P, 8, HID]), op=OP.mult,
                )
                e8 = gap.tile([P, 8], F32, tag="e8")
                nc.vector.tensor_reduce(out=e8[:], in_=z[:], axis=AX.X, op=OP.add)
                w8b = gap.tile([P, 8], BF16, tag="w8b")
                nc.scalar.activation(out=w8b[:], in_=e8[:], func=AF.Exp)
                pay = gap.tile([P, 8, HID], BF16, tag="pay")
                nc.vector.tensor_tensor(
                    out=pay[:], in0=gxs[:, :, 0:HID],
                    in1=w8b[:, :, None].to_broadcast([P, 8, HID]), op=OP.mult,
                )
                for cl in range(8):
                    c = b * 8 + cl
                    gc = w * CPW + c
                    db, pos = gc // SPD, gc % SPD
                    nc.tensor.matmul(
                        nmps[db // 8][:, db % 8, :],
                        lhsT=oh8[:, cl, :], rhs=pay[:, cl, :],
                        start=(pos == 0), stop=(pos == SPD - 1),
                    )
                    nc.tensor.matmul(
                        dnps[:, db : db + 1],
                        lhsT=oh8[:, cl, :], rhs=w8b[:, cl : cl + 1],
                        start=(pos == 0), stop=(pos == SPD - 1),
                    )

        # ---- finalize graph g: h2 = leaky(numer/denom + b_gat) ----
        numsb = gfin.tile([P, TJ, HID], F32, tag="numsb", name=f"numsb{g}")
        nc.scalar.copy(out=numsb[:, 0:8, :], in_=nmps[0][:])
        nc.scalar.copy(out=numsb[:, 8:TJ, :], in_=nmps[1][:])
        den = gfin.tile([P, TJ], F32, tag="den", name=f"den{g}")
        rec = gfin.tile([P, TJ], F32, tag="rec", name=f"rec{g}")
        dtmp = gfin.tile([P, TJ], F32, tag="dtmp", name=f"dtmp{g}")
        nc.vector.tensor_scalar(
            out=den[:], in0=dnps[:], scalar1=1e-16, scalar2=None, op0=OP.add
        )
        nc.vector.reciprocal(out=rec[:], in_=den[:])
        recip_newton(rec[:], den[:], dtmp[:])
        hslc = h2t[:, g * TJ : (g + 1) * TJ, :]
        nc.vector.tensor_tensor(
            out=hslc, in0=numsb[:],
            in1=rec[:, :, None].to_broadcast([P, TJ, HID]), op=OP.mult,
        )
        nc.vector.tensor_tensor(
            out=hslc, in0=hslc,
            in1=bgat_rep[:, None, :].to_broadcast([P, TJ, HID]), op=OP.add,
        )
        nc.vector.scalar_tensor_tensor(
            out=hslc, in0=hslc, scalar=0.01, in1=hslc, op0=OP.mult, op1=OP.max
        )

    phase_a.close()

    # ======== score2 (t-space), mask to kept1 ========
    score2_t = mpool.tile([P, NT], F32, tag="score2_t")
    s2prod = mpool.tile([P, NT, HID], F32, tag="bigC2", name="s2prod")
    nc.vector.tensor_tensor(
        out=s2prod[:], in0=h2t[:],
        in1=p2_rep[:, None, :].to_broadcast([P, NT, HID]), op=OP.mult,
    )
    nc.vector.tensor_reduce(out=score2_t[:], in_=s2prod[:], axis=AX.X, op=OP.add)
    kept1_t = mpool.tile([P, NT], U8, tag="kept1_t")
    nc.vector.tensor_tensor(out=kept1_t[:], in0=gate1[:], in1=gate1[:], op=OP.is_equal)
    sc2m = mpool.tile([P, NT], F32, tag="sc2m")
    nc.vector.tensor_copy(out=sc2m[:], in_=negbig[:])
    nc.vector.copy_predicated(out=sc2m[:], mask=kept1_t[:], data=score2_t[:])

    # ======== pool2 threshold + gate2 = tanh * mask ========
    t2 = bisect_threshold(sc2m, cfg.k2, "p2")
    gate2 = mpool.tile([P, NT], F32, tag="gate2")
    nc.vector.tensor_tensor(
        out=gate2[:].rearrange("p (g t) -> p g t", g=NG),
        in0=sc2m[:].rearrange("p (g t) -> p g t", g=NG),
        in1=t2[:, :, None].to_broadcast([P, NG, TJ]), op=OP.is_gt,
    )
    tanh2 = mpool.tile([P, NT], F32, tag="tanh2")
    sc2c = mpool.tile([P, NT], F32, tag="sc2c")
    nc.vector.tensor_scalar(
        out=sc2c[:], in0=sc2m[:], scalar1=-64.0, scalar2=None, op0=OP.max
    )
    nc.scalar.activation(out=tanh2[:], in_=sc2c[:], func=AF.Tanh)
    nc.vector.tensor_tensor(out=gate2[:], in0=gate2[:], in1=tanh2[:], op=OP.mult)

    # ======== T_g = sum_n gate2[n] * h2[n]; out = T @ W23 + C ========
    Tps = ppool.tile([P, NG], F32, tag="pp2")
    for j in range(NT):
        g = j // TJ
        nc.tensor.matmul(
            Tps[:HID, g : g + 1], lhsT=h2t[:, j, :], rhs=gate2[:, j : j + 1],
            start=(j % TJ == 0), stop=(j % TJ == TJ - 1),
        )
    Tsb = mpool.tile([HID, NG], F32, tag="Tsb")
    nc.scalar.copy(out=Tsb[:], in_=Tps[:HID, :])
    hps2 = ppool.tile([NG, 1], F32, tag="pp")
    nc.tensor.matmul(hps2[:], lhsT=Tsb[:], rhs=Vs["W23"][:], start=True, stop=True)
    outsb = mpool.tile([NG, 1], F32, tag="outsb")
    nc.vector.tensor_tensor(out=outsb[:], in0=hps2[:], in1=Cc_sb[:], op=OP.add)
    nc.sync.dma_start(out=out_d[:], in_=outsb[:])


# ================= host side =================

def _wrap_idx(ix: np.ndarray) -> np.ndarray:
    n = ix.shape[0]
    w = ix.reshape(n // 16, 16).T.astype(np.int16)
    return np.tile(w, (8, 1)).copy()


def _prep_weights(cfg, W1, b1, bn_gamma, bn_beta, bn_mean, bn_var, W_lin1, b_lin1,
                  p1, Wl, Wr, att, b_gat, p2, W_lin2, b_lin2, W_lin3, b_lin3):
    f32 = np.float32
    bn_a = (bn_gamma / np.sqrt(bn_var + 1e-5)).astype(f32)
    bn_b = (bn_beta - bn_mean * bn_a).astype(f32)
    W23 = (W_lin2 @ W_lin3).reshape(-1).astype(f32)
    Cc = np.array([cfg.k2 * float(b_lin2 @ W_lin3[:, 0]) + float(b_lin3[0])], dtype=f32)
    return {
        "W1": np.ascontiguousarray(W1, f32), "Wlin1": np.ascontiguousarray(W_lin1, f32),
        "Wl": np.ascontiguousarray(Wl, f32), "Wr": np.ascontiguousarray(Wr, f32),
        "b1": np.ascontiguousarray(b1, f32), "bn_a": bn_a, "bn_b": bn_b,
        "b_lin1": np.ascontiguousarray(b_lin1, f32),
        "p1": (np.asarray(p1) / np.linalg.norm(np.asarray(p1))).astype(f32),
        "att": np.ascontiguousarray(att, f32), "b_gat": np.ascontiguousarray(b_gat, f32),
        "p2": (np.asarray(p2) / np.linalg.norm(np.asarray(p2))).astype(f32),
        "W23": W23, "Cc": Cc,
    }


def _prep_core_edges(cfg: Cfg, src_core, dst_core):
    """src/dst core-local [ne]. Per graph: append self loops, bucket edges by
    dst block (db = dst>>7), pad each db run to spd*128 slots. Pad slots get
    src=0 (any valid row; killed by the one-hot) and dlo=255 (matches no
    iota value -> all-zero one-hot row/column)."""
    SPD, SRUN = cfg.spd, cfg.spd * P
    loops = np.arange(cfg.npg, dtype=np.int64)
    src_slots = np.zeros((cfg.ng, cfg.tj, SRUN), np.int64)
    dlo_slots = np.full((cfg.ng, cfg.tj, SRUN), 255, np.int64)
    deg = np.zeros((cfg.ng, cfg.npg), np.int64)
    for g in range(cfg.ng):
        e = slice(g * cfg.eg, (g + 1) * cfg.eg)
        s = np.concatenate([src_core[e] - g * cfg.npg, loops])
        d = np.concatenate([dst_core[e] - g * cfg.npg, loops])
        deg[g] = np.bincount(d, minlength=cfg.npg)
        db = d >> 7
        for b in range(cfg.tj):
            m = db == b
            cnt = int(m.sum())
            assert cnt <= SRUN, f"db run overflow: {cnt} > {SRUN}"
            src_slots[g, b, :cnt] = s[m] + g * cfg.npg
            dlo_slots[g, b, :cnt] = d[m] & 127
    stream_src = src_slots.reshape(-1)
    stream_dlo = dlo_slots.reshape(-1)
    deg_t = np.ascontiguousarray(
        deg.reshape(cfg.ng, cfg.tj, P).transpose(2, 0, 1).reshape(P, cfg.nt)
    ).astype(np.float32)
    bf16 = ml_dtypes.bfloat16
    return {
        "srcw": _wrap_idx(stream_src),
        "dlo_pm": np.ascontiguousarray(
            stream_dlo.reshape(-1, P).T.astype(bf16)
        ),
        "dlo_fr": np.ascontiguousarray(dlo_slots.reshape(-1).astype(bf16)),
        "degT": deg_t,
    }


def build_bass(cfg: Cfg):
    from contextlib import ExitStack
    nc = bacc.Bacc("TRN2", target_bir_lowering=False, debug=False,
                   num_swdge_queues=4)
    with tile.TileContext(nc) as tc:
        with ExitStack() as ctx:
            build_core_program(ctx, tc, cfg)
    nc.compile()
    return nc


_CFG = Cfg()
_NC_CACHE = {}
TRACE = False
LAST_RESULT = None


def kernel(x, edge_index, batch, W1, b1, bn_gamma, bn_beta, bn_mean, bn_var,
           W_lin1, b_lin1, p1, Wl, Wr, att, b_gat, p2,
           W_lin2, b_lin2, W_lin3, b_lin3):
    cfg = _CFG
    n_cores = 8
    s_att = float(np.sum(np.asarray(att, dtype=np.float64)))
    assert abs(s_att) > 1e-6, "degenerate att sum; poison scheme needs |sum(att)|>0"
    cfg.psign = -1.0 if s_att > 0 else 1.0
    slope = 0.2 if s_att > 0 else 1.0
    cfg.pb_mag = 40.0 / (slope * abs(s_att))
    weights = _prep_weights(cfg, W1, b1, bn_gamma, bn_beta, bn_mean, bn_var,
                            W_lin1, b_lin1, p1, Wl, Wr, att, b_gat, p2,
                            W_lin2, b_lin2, W_lin3, b_lin3)
    src_all = np.asarray(edge_index[0], dtype=np.int64)
    dst_all = np.asarray(edge_index[1], dtype=np.int64)
    x = np.asarray(x, dtype=np.float32)

    # choose the chunks-per-db-run capacity from the data (global max so the
    # single SPMD program fits every core)
    max_run = 0
    for c in range(n_cores):
        for g in range(cfg.ng):
            e0 = c * cfg.ne + g * cfg.eg
            d = dst_all[e0 : e0 + cfg.eg] - (c * cfg.nn + g * cfg.npg)
            cnts = np.bincount(d >> 7, minlength=cfg.tj) + P  # + self loops
            max_run = max(max_run, int(cnts.max()))
    cfg.spd = (max_run + P - 1) // P
    # windows of ch slots must tile a graph's slot range exactly
    while (cfg.tj * cfg.spd * P) % cfg.ch != 0:
        cfg.spd += 1

    in_maps = []
    for c in range(n_cores):
        n0 = c * cfg.nn
        e0 = c * cfg.ne
        d = dict(weights)
        d.update(
            _prep_core_edges(
                cfg, src_all[e0 : e0 + cfg.ne] - n0, dst_all[e0 : e0 + cfg.ne] - n0
            )
        )
        d["xT"] = np.ascontiguousarray(x[n0 : n0 + cfg.nn].T, np.float32)
        in_maps.append(d)

    key = ("nc", cfg.spd, cfg.psign, cfg.pb_mag)
    if key not in _NC_CACHE:
        _NC_CACHE[key] = build_bass(cfg)
    nc = _NC_CACHE[key]
    global LAST_RESULT
    res = run_bass_kernel_spmd(nc, in_maps, core_ids=list(range(n_cores)), trace=TRACE)
    LAST_RESULT = res
    outs = [np.asarray(res.results[c]["out"]).reshape(cfg.ng, 1) for c in range(n_cores)]
    return np.concatenate(outs, axis=0).astype(np.float32)
